# revision 81
# baseline (speedup 1.0000x reference)
"""Trainium2 8-core kernel for MemoryEfficientAttention.

Full multi-head attention layer: Q/K/V projections + exact-softmax attention
+ output projection for [B=4, S=2048, D=1024], H=16 heads, dk=64.

Sharding: core c handles batch c//2 and head-half c%2 (8 heads = 512 dims).
Each core produces a partial out-projection [2048, 1024] in bf16; the host
sums the two partials per batch (fp32) and adds the output bias.

Optimizations over the original baseline (~503us -> ~417us):
- host stages all inputs/weights in DMA-friendly per-partition-contiguous
  blocks (xq/xk as 4 token segments, xv as 8 chunks, weights/biases
  pre-transposed) so every input DMA uses large descriptors.
- DMA queue plan measured against the HW arbiter: the scalar queue gets
  strict service priority, so it carries the prologue-critical slice
  (wk, xk0, wq, xq0, xq1, wv) in need order; gpsimd (independent SW
  descriptor-gen) carries the small latency-critical SBUF moves (kstg
  scatters, qdup staging, softmax epilogue bounce); sync carries the
  deferred bulk (xk1-3, xq2-3, wo, y).
- attention starts as soon as K(jt0)+Q(jt0,qh0) are projected (~30us);
  V projection and remaining K/Q segs run as deadline-scheduled fillers
  (one ~1.7us item per kt-pair step, popped at the END of the step body).
- the kt-pair loop is software-pipelined one step ahead: scores for
  step t+1 are emitted right after the exps of step t, so the next exp
  never waits on attnV/fillers; scores are par-major so a PSUM-rotation
  WAR can't head-block the independent pair.
- separate tiles per input segment and split kh lo/hi pair-range tiles
  keep the tile framework's whole-tile dependencies precise.
- projection bias-adds on DVE (tensor_scalar, per-partition bias AP);
  ScalarE does only exp (the 256 exps at ~1.15us are the pacing engine).
- outproj(0..7) fill the PE-idle late units 10-13; tail is outproj(8..15).

Per-core dataflow (bf16 matmuls, PSUM fp32):
  QT[jt] = Wq[:,jt]^T @ xqT  [128 j, 2048 q]
  KT -> kh pair layout per head [128, 8 kt-pairs, 128]
  V[kt] [128 tok, 8 heads, 64+1] with a ones column (softmax denominator)
  per unit (q-half, head): 16 units, 8 kt-pair steps:
    sT = K_h @ Q_h   [128 k, 1024 q] PSUM (row-split pair via tile_position)
    eT = exp(sT)     ScalarE -> bf16 SBUF
    av += V_aug^T @ eT  [65, 1024] PSUM accumulated over 16 kt
  oc = av[0:64] * (1/av[64])  DVE reciprocal + row-broadcast DMA + DVE mult
  y  = oc^T @ Wo   [2048, 1024] bf16 partial
"""

import numpy as np

import concourse.bass as bass
import concourse.mybir as mybir
import concourse.tile as tile
from concourse import bacc

B, S, D, H, DK = 4, 2048, 1024, 16, 64
NCORES = 8
HPC = H // 2          # heads per core
DH = HPC * DK         # 512 projection dims per core
NJT = 4               # head pairs per core
NDT = D // 128        # 8 d-tiles
NKT = S // 128        # 16 k-tiles
F32 = mybir.dt.float32
BF16 = mybir.dt.bfloat16
U16 = mybir.dt.uint16
EXP = mybir.ActivationFunctionType.Exp
MULT = mybir.AluOpType.mult
ADD = mybir.AluOpType.add

# pair-Schraudolph constants (bf16 bit trick; dormant unless USE_SCH)
SCH_A = 128.0 / np.log(2.0)
SCH_D = np.log(2.0) / 4.0
SCH_C = 0.3
SCH_B1 = 16256.0 - SCH_C - SCH_A * SCH_D
SCH_B2 = 16256.0 - SCH_C + SCH_A * SCH_D
EXP_BIAS = 0.743598

USE_SCH = False


def _sch_tile(hh, kt):
    return USE_SCH and kt % 8 == 3


def _bcast_rows(ap_row, nrows):
    """AP that reads one partition row `nrows` times (partition step 0)."""
    return bass.AP(
        tensor=ap_row.tensor,
        offset=ap_row.offset,
        ap=[[0, nrows]] + [list(x) for x in ap_row.ap[1:]],
    )


def _emit(nc, tc, ctx):
    xq = nc.dram_tensor("xq", [4, 128, NDT, 512], BF16,
                        kind="ExternalInput").ap()
    xk = nc.dram_tensor("xk", [4, 128, NDT, 512], BF16,
                        kind="ExternalInput").ap()
    xv = nc.dram_tensor("xv", [8, 128, NDT, 256], BF16,
                        kind="ExternalInput").ap()
    wq = nc.dram_tensor("wq", [128, NDT, DH], BF16, kind="ExternalInput").ap()
    wk = nc.dram_tensor("wk", [128, NDT, DH], BF16, kind="ExternalInput").ap()
    wv = nc.dram_tensor("wv", [128, NDT, DH], BF16, kind="ExternalInput").ap()
    wo = nc.dram_tensor("wo", [128, NJT, D], BF16, kind="ExternalInput").ap()
    bq = nc.dram_tensor("bq", [128, NJT], F32, kind="ExternalInput").ap()
    bk = nc.dram_tensor("bk", [128, NJT], F32, kind="ExternalInput").ap()
    bv = nc.dram_tensor("bv", [128, DH], F32, kind="ExternalInput").ap()
    y = nc.dram_tensor("y", [S, D], BF16, kind="ExternalOutput").ap()

    consts = ctx.enter_context(tc.tile_pool(name="consts", bufs=1))
    wpool = ctx.enter_context(tc.tile_pool(name="weights", bufs=2))
    xvpool = ctx.enter_context(tc.tile_pool(name="xvchunks", bufs=3))
    xqkpool = ctx.enter_context(tc.tile_pool(name="xqk", bufs=1))
    qkpool = ctx.enter_context(tc.tile_pool(name="qk", bufs=1))
    vpool = ctx.enter_context(tc.tile_pool(name="vps", bufs=1))
    ocpool = ctx.enter_context(tc.tile_pool(name="ocp", bufs=1))
    epool = ctx.enter_context(tc.tile_pool(name="expt", bufs=3))
    smalls = ctx.enter_context(tc.tile_pool(name="smalls", bufs=2))
    ypool = ctx.enter_context(tc.tile_pool(name="ystage", bufs=2))
    dramp = ctx.enter_context(tc.tile_pool(name="drams", bufs=1, space="DRAM"))
    psum = ctx.enter_context(tc.tile_pool(name="psum", bufs=1, space="PSUM"))
    if USE_SCH:
        upool = ctx.enter_context(tc.tile_pool(name="u16", bufs=2))

    # PSUM: tag s = [128,1024] x3 (6 banks; scores rotate + proj/outproj
    # fillers borrow), av = [128,1024] (2 banks)
    def ps_s(name):
        return psum.tile([128, 1024], F32, tag="s", name=name, bufs=3)

    def ps_av(name):
        return psum.tile([128, 1024], F32, tag="av", name=name, bufs=1)

    def ps_aux(name):
        return psum.tile([128, 1024], F32, tag="s", name=name, bufs=3)[:, 0:512]

    ebias = consts.tile([128, 1], F32)
    nc.vector.memset(ebias, EXP_BIAS if USE_SCH else 0.0)

    qt_t = [qkpool.tile([128, S], BF16, tag=f"q{jt}", name=f"qT{jt}")
            for jt in range(NJT)]
    # K pair layout, per head: [128, 4 kt-pairs, 128]; rows 0:64 = even
    # k-tile, rows 64:128 = odd k-tile (row-split pair scores). Split into
    # lo (pi 0-3) / hi (pi 4-7) tiles so a k_seg filler writing the hi half
    # doesn't serialize the current unit's lo-half scores (whole-tile deps).
    kh_lo = [qkpool.tile([128, NKT // 4, 128], BF16, tag=f"kl{h}",
                         name=f"khl{h}") for h in range(HPC)]
    kh_hi = [qkpool.tile([128, NKT // 4, 128], BF16, tag=f"kh{h}",
                         name=f"khh{h}") for h in range(HPC)]
    v_t = [vpool.tile([128, HPC, DK + 2], BF16, tag=f"v{kt}", name=f"v{kt}")
           for kt in range(NKT)]
    oc_t = [ocpool.tile([128, S], BF16, tag=f"oc{jt}", name=f"oc{jt}")
            for jt in range(NJT)]

    # ---- critical-path DMAs. The HW descriptor-gen serves the scalar
    # queue with strict priority over sync (gpsimd runs its own SW DGE),
    # so: critical slice on scalar+gpsimd in need order, deferred bulk on
    # sync, and all small latency-critical SBUF moves (kstg/qdup/y/
    # epilogue) on gpsimd whose engine is otherwise idle.
    xk_sb = [xqkpool.tile([128, NDT, 512], BF16, tag=f"xk{s}",
                          name=f"xk_sb{s}") for s in range(4)]
    xq_sb = [xqkpool.tile([128, NDT, 512], BF16, tag=f"xq{s}",
                          name=f"xq_sb{s}") for s in range(4)]

    wk_sb = wpool.tile([128, NDT, DH], BF16, tag="wqk", name="w_k", bufs=1)
    nc.scalar.dma_start(out=wk_sb, in_=wk)
    nc.scalar.dma_start(out=xk_sb[0], in_=xk[0])
    wq_sb = wpool.tile([128, NDT, DH], BF16, tag="wqk2", name="w_q", bufs=1)
    nc.scalar.dma_start(out=wq_sb, in_=wq)
    nc.scalar.dma_start(out=xq_sb[0], in_=xq[0])
    nc.scalar.dma_start(out=xq_sb[1], in_=xq[1])
    wv_sb = wpool.tile([128, NDT, DH], BF16, tag="w", name="w_v", bufs=1)
    nc.scalar.dma_start(out=wv_sb, in_=wv)

    bq_sb = consts.tile([128, NJT], F32)
    nc.sync.dma_start(out=bq_sb, in_=bq)
    bk_sb = consts.tile([128, NJT], F32)
    nc.sync.dma_start(out=bk_sb, in_=bk)
    bv_row = consts.tile([128, DH], F32)
    nc.sync.dma_start(out=bv_row, in_=bv)
    nc.sync.dma_start(out=xk_sb[1], in_=xk[1])

    # ---- V projection per 256-token chunk (kt pair) ----
    def v_chunk(c):
        def emit():
            xc = xvpool.tile([128, NDT, 256], BF16, tag="xv", name=f"xv{c}",
                             bufs=2)
            nc.gpsimd.dma_start(out=xc, in_=xv[c])
            for ktl in range(2):
                kt = 2 * c + ktl
                pv = ps_aux(f"pv{kt}")
                for dt in range(NDT):
                    nc.tensor.matmul(
                        pv[:],
                        lhsT=xc[:, dt, ktl * 128:(ktl + 1) * 128],
                        rhs=wv_sb[:, dt, :],
                        start=(dt == 0),
                        stop=(dt == NDT - 1),
                    )
                vt = v_t[kt]
                nc.vector.memset(vt[:, :, DK:DK + 1], 1.0)
                nc.vector.tensor_add(
                    out=vt[:, :, 0:DK],
                    in0=pv.rearrange("p (h d) -> p h d", h=HPC),
                    in1=bv_row.rearrange("p (h d) -> p h d", h=HPC),
                )
        return emit

    # ---- K projection per (jt, seg): 512 k-tokens -> kh pair layout ----
    def k_seg(jt, seg):
        def emit():
            pq = ps_aux(f"pk{jt}{seg}")
            for dt in range(NDT):
                nc.tensor.matmul(
                    pq[:],
                    lhsT=wk_sb[:, dt, jt * 128:(jt + 1) * 128],
                    rhs=xk_sb[seg][:, dt, :],
                    start=(dt == 0),
                    stop=(dt == NDT - 1),
                )
            stage = smalls.tile([128, 4, 128], BF16, tag="kstg",
                                name="kstg", bufs=2)
            nc.vector.tensor_scalar(
                out=stage[:], in0=pq.rearrange("p (a b) -> p a b", a=4),
                scalar1=bk_sb[:, jt:jt + 1], scalar2=None, op0=ADD)
            kh = kh_lo if seg < 2 else kh_hi
            sg = seg % 2
            for hh in range(2):
                h = 2 * jt + hh
                r0 = hh * 64
                for par in range(2):   # kt parity within the seg
                    nc.gpsimd.dma_start(
                        out=kh[h][par * 64:par * 64 + 64,
                                  2 * sg:2 * sg + 2, :],
                        in_=stage[r0:r0 + 64, par::2, :],
                    )
        return emit

    def k_segs(jt, segs):
        def emit():
            for seg in segs:
                k_seg(jt, seg)()
        return emit

    # ---- Q projection per (jt, seg) ----
    def q_seg(jt, seg):
        def emit():
            pq = ps_aux(f"pq{jt}{seg}")
            for dt in range(NDT):
                nc.tensor.matmul(
                    pq[:],
                    lhsT=wq_sb[:, dt, jt * 128:(jt + 1) * 128],
                    rhs=xq_sb[seg][:, dt, :],
                    start=(dt == 0),
                    stop=(dt == NDT - 1),
                )
            nc.vector.tensor_scalar(
                out=qt_t[jt][:, seg * 512:(seg + 1) * 512], in0=pq[:],
                scalar1=bq_sb[:, jt:jt + 1], scalar2=None, op0=ADD)
        return emit

    def q_segs(jt, segs):
        def emit():
            for seg in segs:
                q_seg(jt, seg)()
        return emit

    # ---- out-projection closure (one q-row-tile) ----
    def outproj(qt):
        def emit():
            py = psum.tile([128, 1024], F32, tag="s", name=f"py{qt}", bufs=3)
            for nb in range(2):
                for jt in range(NJT):
                    nc.tensor.matmul(
                        py[:, nb * 512:(nb + 1) * 512],
                        lhsT=oc_t[jt][:, qt * 128:(qt + 1) * 128],
                        rhs=wo_sb[:, jt, nb * 512:(nb + 1) * 512],
                        start=(jt == 0),
                        stop=(jt == NJT - 1),
                    )
            ys = ypool.tile([128, 1024], BF16, tag="y", name="ys", bufs=2)
            nc.vector.tensor_copy(out=ys[:], in_=py[:])
            nc.sync.dma_start(out=y[qt * 128:(qt + 1) * 128, :], in_=ys[:])
        return emit

    # ---- qdup staging: duplicate a head's Q rows into both halves ----
    def qdup_for(jt, qh, hh):
        q0 = qh * 1024
        r0 = hh * 64
        # the unit's Q rows already sit natively in partition half `hh`;
        # only the opposite half needs a staged copy for the other
        # contraction row-tile.
        qd = smalls.tile([128, 1024], BF16, tag="qh", name="qdup", bufs=2)
        oh = (1 - hh) * 64
        nc.gpsimd.dma_start(out=qd[oh:oh + 64, :],
                            in_=qt_t[jt][r0:r0 + 64, q0:q0 + 1024])
        return qd

    units = [(qh, jt, hh) for qh in range(2) for jt in range(NJT)
             for hh in range(2)]
    tasks = [(ui, pi) for ui in range(len(units)) for pi in range(NKT // 2)]
    qdups = {}
    avps = {}
    sts = {}

    def emit_scores(ui, pi):
        qh_, jt_, hh_ = units[ui]
        h_ = 2 * jt_ + hh_
        q0_ = qh_ * 1024
        sTs = [ps_s(f"sT{ui}_{2 * pi + p}") for p in range(2)]
        kh = kh_lo[h_] if pi < 4 else kh_hi[h_]
        qdup = qdups[ui]
        # alternate the two row-split tiles so their streams overlap in
        # the PE array (same-tile matmuls back-to-back would serialize
        # and head-of-line-block the independent tile behind them).
        # the row-tile matching this head's native half reads qt_t
        # directly; the other half reads the staged copy.
        for qbh in range(2):
            for par in range(2):
                if par == hh_:
                    rhs = qt_t[jt_][hh_ * 64:hh_ * 64 + 64,
                                    q0_ + qbh * 512:q0_ + (qbh + 1) * 512]
                else:
                    rhs = qdup[par * 64:par * 64 + 64,
                               qbh * 512:(qbh + 1) * 512]
                nc.tensor.matmul(
                    sTs[par][:, qbh * 512:(qbh + 1) * 512],
                    lhsT=kh[par * 64:par * 64 + 64, pi % 4, :],
                    rhs=rhs,
                    start=True,
                    stop=True,
                    tile_position=(par * 64, 0),
                )
        return sTs

    # ---- prologue: minimum work before unit 0's first scores; the
    # first scores are emitted BEFORE the V chunks so the first exp
    # doesn't sit behind them in the PE program. ----
    k_seg(0, 0)()
    k_seg(0, 1)()
    q_seg(0, 0)()
    q_seg(0, 1)()
    qdups[0] = qdup_for(0, 0, 0)
    sts[(0, 0)] = emit_scores(0, 0)
    v_chunk(0)()
    v_chunk(1)()
    # deferred input DMAs (not needed for the first unit's early steps)
    nc.sync.dma_start(out=xk_sb[2], in_=xk[2])
    nc.sync.dma_start(out=xk_sb[3], in_=xk[3])
    nc.sync.dma_start(out=xq_sb[2], in_=xq[2])
    nc.sync.dma_start(out=xq_sb[3], in_=xq[3])
    wo_sb = wpool.tile([128, NJT, D], BF16, tag="wo", name="w_o", bufs=1)
    nc.sync.dma_start(out=wo_sb, in_=wo)

    # filler schedule: per unit, list of (pi, closure); popped at the END
    # of the kt-pair body whose pi matches (so they never delay scores).
    fillers = {
        0: [(0, v_chunk(2)), (0, k_seg(0, 2)), (1, v_chunk(3)),
            (2, v_chunk(4)), (2, k_seg(0, 3)), (3, v_chunk(5)),
            (4, v_chunk(6)), (5, v_chunk(7))],
        1: [(0, k_seg(1, 0)), (1, k_seg(1, 1)), (2, q_seg(1, 0)),
            (3, q_seg(1, 1)), (4, k_seg(1, 2)), (5, k_seg(1, 3))],
        2: [(0, k_seg(2, 0)), (2, k_seg(2, 1)), (4, q_seg(2, 0)),
            (6, q_seg(2, 1))],
        3: [(0, k_seg(2, 2)), (2, k_seg(2, 3)), (4, k_seg(3, 0)),
            (6, k_seg(3, 1))],
        4: [(0, k_seg(3, 2)), (2, k_seg(3, 3)), (4, q_seg(3, 0)),
            (6, q_seg(3, 1))],
        5: [(0, q_seg(0, 2)), (2, q_seg(0, 3))],
        6: [(0, q_seg(1, 2)), (2, q_seg(1, 3))],
        7: [(0, q_seg(2, 2)), (2, q_seg(2, 3))],
        8: [(0, q_seg(3, 2)), (2, q_seg(3, 3))],
        9: [],
        10: [(1, outproj(0)), (5, outproj(1))],
        11: [(1, outproj(2)), (5, outproj(3))],
        12: [(1, outproj(4)), (5, outproj(5))],
        13: [(1, outproj(6)), (5, outproj(7))],
        14: [], 15: [],
    }

    # ---- attention: 16 units (q-half, head-pair, head), software-
    # pipelined one kt-pair ahead: scores for step t+1 are emitted right
    # after the exps of step t, so the next exp never waits for attnV.
    todos = {ui: list(fillers[ui]) for ui in range(len(units))}
    for ti, (ui, pi) in enumerate(tasks):
        qh, jt, hh = units[ui]
        q0 = qh * 1024
        r0 = hh * 64
        h = 2 * jt + hh
        if pi == 0:
            avps[ui] = ps_av(f"av{ui}")
        avp = avps[ui]
        sTs = sts.pop((ui, pi))
        et2 = epool.tile([128, 2048], BF16, tag="e", name="et", bufs=3)
        ets = [et2[:, 0:1024], et2[:, 1024:2048]]
        a0, a1 = sTs[0][:], sTs[1][:]
        if (a1.offset - a0.offset) == a0.ap[1][0] * 1024:
            # the two score buffers are adjacent in PSUM: one N=2048
            # activation covers both k-tiles (saves the ~290ns ACT ramp)
            merged = bass.AP(tensor=a0.tensor, offset=a0.offset,
                             ap=[list(a0.ap[0]), [a0.ap[1][0], 2048]])
            nc.scalar.activation(et2[:], merged, EXP, bias=ebias)
        else:
            for par in range(2):
                nc.scalar.activation(ets[par], sTs[par][:], EXP, bias=ebias)
        # prefetch next unit's qdup mid-unit so its scores never wait
        if pi == 3 and ui + 1 < len(units):
            nqh, njt, nhh = units[ui + 1]
            qdups[ui + 1] = qdup_for(njt, nqh, nhh)
        # scores one step ahead (possibly crossing into the next unit)
        if ti + 1 < len(tasks):
            nui, npi = tasks[ti + 1]
            sts[(nui, npi)] = emit_scores(nui, npi)
        for par in range(2):
            kt = 2 * pi + par
            for qbh in range(2):
                nc.tensor.matmul(
                    avp[0:DK + 1, qbh * 512:(qbh + 1) * 512],
                    lhsT=v_t[kt][:, h, 0:DK + 1],
                    rhs=ets[par][:, qbh * 512:(qbh + 1) * 512],
                    start=(kt == 0),
                    stop=(kt == NKT - 1),
                )
        todo = todos[ui]
        if pi < NKT // 2 - 1:
            while todo and todo[0][0] <= pi:
                todo.pop(0)[1]()
            continue
        # ---- end of unit: epilogue first (its avsb copy releases avp
        # for the next unit), then leftover fillers ----
        avsb = smalls.tile([128, 1024], F32, tag="avsb", name="avsb", bufs=1)
        nc.vector.tensor_copy(out=avsb[DK:DK + 1, :], in_=avp[DK:DK + 1, :])
        rsb = smalls.tile([128, 16], F32, tag="rsb", name="rsb", bufs=2)
        nc.gpsimd.dma_start(out=rsb[0:64, :], in_=avsb[DK:DK + 1, :])
        nc.vector.tensor_copy(out=avsb[0:DK, :], in_=avp[0:DK, :])
        rc2 = smalls.tile([128, 16], F32, tag="rc2", name="rc2", bufs=2)
        nc.vector.reciprocal(rc2[0:64, :], rsb[0:64, :])
        rdram = dramp.tile([1024], F32, tag="rdram", name="rdram", bufs=2)
        nc.gpsimd.dma_start(out=rdram.rearrange("(p a) -> p a", p=64),
                            in_=rc2[0:64, :])
        rb = smalls.tile([128, 1024], F32, tag="rb", name="rb", bufs=1)
        nc.gpsimd.dma_start(out=rb[0:DK, :],
                            in_=_bcast_rows(rdram[None, :], DK))
        nc.vector.scalar_tensor_tensor(
            out=oc_t[jt][r0:r0 + 64, q0:q0 + 1024],
            in0=avsb[0:DK, :], scalar=1.0, in1=rb[0:DK, :],
            op0=MULT, op1=MULT)
        while todo:
            todo.pop(0)[1]()

    # ---- remaining out-projection (qh1 rows) ----
    for qt in range(8, NKT):
        outproj(qt)()


_CACHE = {}


def _build():
    if "nc" in _CACHE:
        return _CACHE["nc"]
    from contextlib import ExitStack

    nc = bacc.Bacc("TRN2", target_bir_lowering=False, debug=False,
                   num_devices=NCORES)
    with tile.TileContext(nc) as tc:
        with ExitStack() as ctx:
            _emit(nc, tc, ctx)
    nc.compile()
    _CACHE["nc"] = nc
    return nc


def _stage_segs(xT, nseg, tseg):
    """[D, S] (d = n*128+p) -> [nseg, 128p, 8n, tseg] contiguous."""
    A = xT.reshape(NDT, 128, nseg, tseg)
    return np.ascontiguousarray(A.transpose(2, 1, 0, 3))


def _stage_w(W):
    """[D, J] (d = n*128+p) -> [128p, n, J] contiguous."""
    A = W.reshape(-1, 128, W.shape[1])
    return np.ascontiguousarray(A.transpose(1, 0, 2))


def make_in_maps(query, key, value, Wq, bq, Wk, bk, Wv, bv, Wo, bo):
    import ml_dtypes
    bf = ml_dtypes.bfloat16
    f32 = np.float32
    query = np.asarray(query, f32)
    key = np.asarray(key, f32)
    value = np.asarray(value, f32)
    Wq, Wk, Wv, Wo = (np.asarray(a, f32) for a in (Wq, Wk, Wv, Wo))
    bq, bk, bv = (np.asarray(a, f32) for a in (bq, bk, bv))
    scale = f32(1.0 / np.sqrt(DK))
    xT = {}
    for b in range(B):
        xT[b] = (
            _stage_segs(query[b].T.astype(bf), 4, 512),
            _stage_segs(key[b].T.astype(bf), 4, 512),
            _stage_segs(value[b].T.astype(bf), 8, 256),
        )
    in_maps = []
    for c in range(NCORES):
        b, hh = divmod(c, 2)
        js = slice(hh * DH, (hh + 1) * DH)
        xqT, xkT, xvT = xT[b]
        in_maps.append({
            "xq": xqT,
            "xk": xkT,
            "xv": xvT,
            "wq": _stage_w((Wq[:, js] * scale).astype(bf)),
            "bq": np.ascontiguousarray((bq[js] * scale).reshape(NJT, 128).T),
            "wk": _stage_w(Wk[:, js].astype(bf)),
            "bk": np.ascontiguousarray(bk[js].reshape(NJT, 128).T),
            "wv": _stage_w(Wv[:, js].astype(bf)),
            "bv": np.ascontiguousarray(np.broadcast_to(bv[js], (128, DH))),
            "wo": _stage_w(Wo[js, :].astype(bf)),
        })
    return in_maps


LAST_RESULTS = None


def kernel(query, key, value, Wq, bq, Wk, bk, Wv, bv, Wo, bo):
    global LAST_RESULTS
    import os
    from concourse.bass_utils import run_bass_kernel_spmd

    nc = _build()
    in_maps = make_in_maps(query, key, value, Wq, bq, Wk, bk, Wv, bv, Wo, bo)
    trace = bool(int(os.environ.get("KERNEL_TRACE", "0")))
    res = run_bass_kernel_spmd(nc, in_maps, list(range(NCORES)), trace=trace)
    LAST_RESULTS = res
    bo32 = np.asarray(bo, dtype=np.float32)
    out = np.empty((B, S, D), dtype=np.float32)
    for b in range(B):
        out[b] = (res.results[2 * b]["y"].astype(np.float32)
                  + res.results[2 * b + 1]["y"].astype(np.float32) + bo32)
    return out


# revision 82
# speedup vs baseline: 1.1591x; 1.1591x over previous
"""Trainium2 8-core kernel for MemoryEfficientAttention.

Full multi-head attention layer: Q/K/V projections + exact-softmax attention
+ output projection for [B=4, S=2048, D=1024], H=16 heads, dk=64.

Sharding: core c handles batch c//2 and head-half c%2 (8 heads = 512 dims).
Each core produces a partial out-projection [2048, 1024] in bf16; the host
sums the two partials per batch (fp32) and adds the output bias.

Optimizations over the original baseline (~503us -> ~417us):
- host stages all inputs/weights in DMA-friendly per-partition-contiguous
  blocks (xq/xk as 4 token segments, xv as 8 chunks, weights/biases
  pre-transposed) so every input DMA uses large descriptors.
- DMA queue plan measured against the HW arbiter: the scalar queue gets
  strict service priority, so it carries the prologue-critical slice
  (wk, xk0, wq, xq0, xq1, wv) in need order; gpsimd (independent SW
  descriptor-gen) carries the small latency-critical SBUF moves (kstg
  scatters, qdup staging, softmax epilogue bounce); sync carries the
  deferred bulk (xk1-3, xq2-3, wo, y).
- attention starts as soon as K(jt0)+Q(jt0,qh0) are projected (~30us);
  V projection and remaining K/Q segs run as deadline-scheduled fillers
  (one ~1.7us item per kt-pair step, popped at the END of the step body).
- the kt-pair loop is software-pipelined one step ahead: scores for
  step t+1 are emitted right after the exps of step t, so the next exp
  never waits on attnV/fillers; scores are par-major so a PSUM-rotation
  WAR can't head-block the independent pair.
- separate tiles per input segment and split kh lo/hi pair-range tiles
  keep the tile framework's whole-tile dependencies precise.
- projection bias-adds on DVE (tensor_scalar, per-partition bias AP);
  ScalarE does only exp (the 256 exps at ~1.15us are the pacing engine).
- outproj(0..7) fill the PE-idle late units 10-13; tail is outproj(8..15).

Per-core dataflow (bf16 matmuls, PSUM fp32):
  QT[jt] = Wq[:,jt]^T @ xqT  [128 j, 2048 q]
  KT -> kh pair layout per head [128, 8 kt-pairs, 128]
  V[kt] [128 tok, 8 heads, 64+1] with a ones column (softmax denominator)
  per unit (q-half, head): 16 units, 8 kt-pair steps:
    sT = K_h @ Q_h   [128 k, 1024 q] PSUM (row-split pair via tile_position)
    eT = exp(sT)     ScalarE -> bf16 SBUF
    av += V_aug^T @ eT  [65, 1024] PSUM accumulated over 16 kt
  oc = av[0:64] * (1/av[64])  DVE reciprocal + row-broadcast DMA + DVE mult
  y  = oc^T @ Wo   [2048, 1024] bf16 partial
"""

import numpy as np

import concourse.bass as bass
import concourse.mybir as mybir
import concourse.tile as tile
from concourse import bacc

B, S, D, H, DK = 4, 2048, 1024, 16, 64
NCORES = 8
HPC = H // 2          # heads per core
DH = HPC * DK         # 512 projection dims per core
NJT = 4               # head pairs per core
NDT = D // 128        # 8 d-tiles
NKT = S // 128        # 16 k-tiles
F32 = mybir.dt.float32
BF16 = mybir.dt.bfloat16
U16 = mybir.dt.uint16
EXP = mybir.ActivationFunctionType.Exp
MULT = mybir.AluOpType.mult
ADD = mybir.AluOpType.add

# pair-Schraudolph constants (bf16 bit trick; dormant unless USE_SCH)
SCH_A = 128.0 / np.log(2.0)
SCH_D = np.log(2.0) / 4.0
SCH_C = 0.3
SCH_B1 = 16256.0 - SCH_C - SCH_A * SCH_D
SCH_B2 = 16256.0 - SCH_C + SCH_A * SCH_D
EXP_BIAS = 0.743598

USE_SCH = False


def _sch_tile(hh, kt):
    return USE_SCH and kt % 8 == 3


def _bcast_rows(ap_row, nrows):
    """AP that reads one partition row `nrows` times (partition step 0)."""
    return bass.AP(
        tensor=ap_row.tensor,
        offset=ap_row.offset,
        ap=[[0, nrows]] + [list(x) for x in ap_row.ap[1:]],
    )


def _emit(nc, tc, ctx):
    xq = nc.dram_tensor("xq", [4, 128, NDT, 512], BF16,
                        kind="ExternalInput").ap()
    xk = nc.dram_tensor("xk", [4, 128, NDT, 512], BF16,
                        kind="ExternalInput").ap()
    xv = nc.dram_tensor("xv", [8, 128, NDT, 256], BF16,
                        kind="ExternalInput").ap()
    wq = nc.dram_tensor("wq", [128, NDT, DH], BF16, kind="ExternalInput").ap()
    wk = nc.dram_tensor("wk", [128, NDT, DH], BF16, kind="ExternalInput").ap()
    wv = nc.dram_tensor("wv", [128, NDT, DH], BF16, kind="ExternalInput").ap()
    wo = nc.dram_tensor("wo", [128, NJT, D], BF16, kind="ExternalInput").ap()
    bq = nc.dram_tensor("bq", [128, NJT], F32, kind="ExternalInput").ap()
    bk = nc.dram_tensor("bk", [128, NJT], F32, kind="ExternalInput").ap()
    bv = nc.dram_tensor("bv", [128, DH], F32, kind="ExternalInput").ap()
    y = nc.dram_tensor("y", [S, D], BF16, kind="ExternalOutput").ap()

    consts = ctx.enter_context(tc.tile_pool(name="consts", bufs=1))
    wpool = ctx.enter_context(tc.tile_pool(name="weights", bufs=2))
    xvpool = ctx.enter_context(tc.tile_pool(name="xvchunks", bufs=3))
    xqkpool = ctx.enter_context(tc.tile_pool(name="xqk", bufs=1))
    qkpool = ctx.enter_context(tc.tile_pool(name="qk", bufs=1))
    vpool = ctx.enter_context(tc.tile_pool(name="vps", bufs=1))
    ocpool = ctx.enter_context(tc.tile_pool(name="ocp", bufs=1))
    epool = ctx.enter_context(tc.tile_pool(name="expt", bufs=3))
    smalls = ctx.enter_context(tc.tile_pool(name="smalls", bufs=2))
    ypool = ctx.enter_context(tc.tile_pool(name="ystage", bufs=2))
    dramp = ctx.enter_context(tc.tile_pool(name="drams", bufs=1, space="DRAM"))
    psum = ctx.enter_context(tc.tile_pool(name="psum", bufs=1, space="PSUM"))
    if USE_SCH:
        upool = ctx.enter_context(tc.tile_pool(name="u16", bufs=2))

    # PSUM: tag s = [128,1024] x3 (6 banks; scores rotate + proj/outproj
    # fillers borrow), av = [128,1024] (2 banks)
    def ps_s(name):
        return psum.tile([128, 1024], F32, tag="s", name=name, bufs=3)

    def ps_av(name):
        return psum.tile([128, 1024], F32, tag="av", name=name, bufs=1)

    def ps_aux(name):
        return psum.tile([128, 1024], F32, tag="s", name=name, bufs=3)[:, 0:512]

    ebias = consts.tile([128, 1], F32)
    nc.vector.memset(ebias, EXP_BIAS if USE_SCH else 0.0)

    qt_t = [qkpool.tile([128, S], BF16, tag=f"q{jt}", name=f"qT{jt}")
            for jt in range(NJT)]
    # K pair layout, per head: [128, 4 kt-pairs, 128]; rows 0:64 = even
    # k-tile, rows 64:128 = odd k-tile (row-split pair scores). Split into
    # lo (pi 0-3) / hi (pi 4-7) tiles so a k_seg filler writing the hi half
    # doesn't serialize the current unit's lo-half scores (whole-tile deps).
    kh_lo = [qkpool.tile([128, NKT // 4, 128], BF16, tag=f"kl{h}",
                         name=f"khl{h}") for h in range(HPC)]
    kh_hi = [qkpool.tile([128, NKT // 4, 128], BF16, tag=f"kh{h}",
                         name=f"khh{h}") for h in range(HPC)]
    v_t = [vpool.tile([128, HPC, DK + 2], BF16, tag=f"v{kt}", name=f"v{kt}")
           for kt in range(NKT)]
    oc_t = [ocpool.tile([128, S], BF16, tag=f"oc{jt}", name=f"oc{jt}")
            for jt in range(NJT)]

    # ---- critical-path DMAs. The HW descriptor-gen serves the scalar
    # queue with strict priority over sync (gpsimd runs its own SW DGE),
    # so: critical slice on scalar+gpsimd in need order, deferred bulk on
    # sync, and all small latency-critical SBUF moves (kstg/qdup/y/
    # epilogue) on gpsimd whose engine is otherwise idle.
    xk_sb = [xqkpool.tile([128, NDT, 512], BF16, tag=f"xk{s}",
                          name=f"xk_sb{s}") for s in range(4)]
    xq_sb = [xqkpool.tile([128, NDT, 512], BF16, tag=f"xq{s}",
                          name=f"xq_sb{s}") for s in range(4)]

    wk_sb = wpool.tile([128, NDT, DH], BF16, tag="wqk", name="w_k", bufs=1)
    nc.scalar.dma_start(out=wk_sb, in_=wk)
    nc.scalar.dma_start(out=xk_sb[0], in_=xk[0])
    wq_sb = wpool.tile([128, NDT, DH], BF16, tag="wqk2", name="w_q", bufs=1)
    nc.scalar.dma_start(out=wq_sb, in_=wq)
    nc.scalar.dma_start(out=xq_sb[0], in_=xq[0])
    nc.scalar.dma_start(out=xq_sb[1], in_=xq[1])
    wv_sb = wpool.tile([128, NDT, DH], BF16, tag="w", name="w_v", bufs=1)
    nc.scalar.dma_start(out=wv_sb, in_=wv)

    bq_sb = consts.tile([128, NJT], F32)
    nc.sync.dma_start(out=bq_sb, in_=bq)
    bk_sb = consts.tile([128, NJT], F32)
    nc.sync.dma_start(out=bk_sb, in_=bk)
    bv_row = consts.tile([128, DH], F32)
    nc.sync.dma_start(out=bv_row, in_=bv)
    nc.sync.dma_start(out=xk_sb[1], in_=xk[1])

    # ---- V projection per 256-token chunk (kt pair) ----
    def v_chunk(c):
        def emit():
            xc = xvpool.tile([128, NDT, 256], BF16, tag="xv", name=f"xv{c}",
                             bufs=2)
            nc.gpsimd.dma_start(out=xc, in_=xv[c])
            for ktl in range(2):
                kt = 2 * c + ktl
                pv = ps_aux(f"pv{kt}")
                for dt in range(NDT):
                    nc.tensor.matmul(
                        pv[:],
                        lhsT=xc[:, dt, ktl * 128:(ktl + 1) * 128],
                        rhs=wv_sb[:, dt, :],
                        start=(dt == 0),
                        stop=(dt == NDT - 1),
                    )
                vt = v_t[kt]
                nc.vector.memset(vt[:, :, DK:DK + 1], 1.0)
                nc.vector.tensor_add(
                    out=vt[:, :, 0:DK],
                    in0=pv.rearrange("p (h d) -> p h d", h=HPC),
                    in1=bv_row.rearrange("p (h d) -> p h d", h=HPC),
                )
        return emit

    # ---- K projection per (jt, seg): 512 k-tokens -> kh pair layout ----
    def k_seg(jt, seg):
        def emit():
            pq = ps_aux(f"pk{jt}{seg}")
            for dt in range(NDT):
                nc.tensor.matmul(
                    pq[:],
                    lhsT=wk_sb[:, dt, jt * 128:(jt + 1) * 128],
                    rhs=xk_sb[seg][:, dt, :],
                    start=(dt == 0),
                    stop=(dt == NDT - 1),
                )
            stage = smalls.tile([128, 4, 128], BF16, tag="kstg",
                                name="kstg", bufs=2)
            nc.vector.tensor_scalar(
                out=stage[:], in0=pq.rearrange("p (a b) -> p a b", a=4),
                scalar1=bk_sb[:, jt:jt + 1], scalar2=None, op0=ADD)
            kh = kh_lo if seg < 2 else kh_hi
            sg = seg % 2
            for hh in range(2):
                h = 2 * jt + hh
                r0 = hh * 64
                for par in range(2):   # kt parity within the seg
                    nc.gpsimd.dma_start(
                        out=kh[h][par * 64:par * 64 + 64,
                                  2 * sg:2 * sg + 2, :],
                        in_=stage[r0:r0 + 64, par::2, :],
                    )
        return emit

    def k_segs(jt, segs):
        def emit():
            for seg in segs:
                k_seg(jt, seg)()
        return emit

    # ---- Q projection per (jt, seg) ----
    def q_seg(jt, seg):
        def emit():
            pq = ps_aux(f"pq{jt}{seg}")
            for dt in range(NDT):
                nc.tensor.matmul(
                    pq[:],
                    lhsT=wq_sb[:, dt, jt * 128:(jt + 1) * 128],
                    rhs=xq_sb[seg][:, dt, :],
                    start=(dt == 0),
                    stop=(dt == NDT - 1),
                )
            nc.vector.tensor_scalar(
                out=qt_t[jt][:, seg * 512:(seg + 1) * 512], in0=pq[:],
                scalar1=bq_sb[:, jt:jt + 1], scalar2=None, op0=ADD)
        return emit

    def q_segs(jt, segs):
        def emit():
            for seg in segs:
                q_seg(jt, seg)()
        return emit

    # ---- out-projection closure (one q-row-tile) ----
    def outproj(qt):
        def emit():
            py = psum.tile([128, 1024], F32, tag="s", name=f"py{qt}", bufs=3)
            for nb in range(2):
                for jt in range(NJT):
                    nc.tensor.matmul(
                        py[:, nb * 512:(nb + 1) * 512],
                        lhsT=oc_t[jt][:, qt * 128:(qt + 1) * 128],
                        rhs=wo_sb[:, jt, nb * 512:(nb + 1) * 512],
                        start=(jt == 0),
                        stop=(jt == NJT - 1),
                    )
            ys = ypool.tile([128, 1024], BF16, tag="y", name="ys", bufs=2)
            nc.vector.tensor_copy(out=ys[:], in_=py[:])
            nc.sync.dma_start(out=y[qt * 128:(qt + 1) * 128, :], in_=ys[:])
        return emit

    # ---- qdup staging: duplicate a head's Q rows into both halves ----
    def qdup_for(jt, qh, hh):
        q0 = qh * 1024
        r0 = hh * 64
        qd = smalls.tile([128, 1024], BF16, tag="qh", name="qdup", bufs=2)
        for half in range(2):
            nc.gpsimd.dma_start(out=qd[half * 64:half * 64 + 64, :],
                                in_=qt_t[jt][r0:r0 + 64, q0:q0 + 1024])
        return qd

    units = [(qh, jt, hh) for qh in range(2) for jt in range(NJT)
             for hh in range(2)]
    tasks = [(ui, pi) for ui in range(len(units)) for pi in range(NKT // 2)]
    qdups = {}
    avps = {}
    sts = {}

    def emit_scores(ui, pi):
        _, jt_, hh_ = units[ui]
        h_ = 2 * jt_ + hh_
        sTs = [ps_s(f"sT{ui}_{2 * pi + p}") for p in range(2)]
        kh = kh_lo[h_] if pi < 4 else kh_hi[h_]
        qdup = qdups[ui]
        # alternate the two row-split tiles so their streams overlap in
        # the PE array (same-tile matmuls back-to-back would serialize
        # and head-of-line-block the independent tile behind them).
        for qbh in range(2):
            for par in range(2):
                nc.tensor.matmul(
                    sTs[par][:, qbh * 512:(qbh + 1) * 512],
                    lhsT=kh[par * 64:par * 64 + 64, pi % 4, :],
                    rhs=qdup[par * 64:par * 64 + 64,
                             qbh * 512:(qbh + 1) * 512],
                    start=True,
                    stop=True,
                    tile_position=(par * 64, 0),
                )
        return sTs

    # ---- prologue: minimum work before unit 0's first scores; the
    # first scores are emitted BEFORE the V chunks so the first exp
    # doesn't sit behind them in the PE program. ----
    k_seg(0, 0)()
    k_seg(0, 1)()
    q_seg(0, 0)()
    q_seg(0, 1)()
    qdups[0] = qdup_for(0, 0, 0)
    sts[(0, 0)] = emit_scores(0, 0)
    v_chunk(0)()
    v_chunk(1)()
    # deferred input DMAs (not needed for the first unit's early steps)
    nc.sync.dma_start(out=xk_sb[2], in_=xk[2])
    nc.sync.dma_start(out=xk_sb[3], in_=xk[3])
    nc.sync.dma_start(out=xq_sb[2], in_=xq[2])
    nc.sync.dma_start(out=xq_sb[3], in_=xq[3])
    wo_sb = wpool.tile([128, NJT, D], BF16, tag="wo", name="w_o", bufs=1)
    nc.sync.dma_start(out=wo_sb, in_=wo)

    # filler schedule: per unit, list of (pi, closure); popped at the END
    # of the kt-pair body whose pi matches (so they never delay scores).
    fillers = {
        0: [(0, v_chunk(2)), (0, k_seg(0, 2)), (1, v_chunk(3)),
            (2, v_chunk(4)), (2, k_seg(0, 3)), (3, v_chunk(5)),
            (4, v_chunk(6)), (5, v_chunk(7))],
        1: [(0, k_seg(1, 0)), (1, k_seg(1, 1)), (2, q_seg(1, 0)),
            (3, q_seg(1, 1)), (4, k_seg(1, 2)), (5, k_seg(1, 3))],
        2: [(0, k_seg(2, 0)), (2, k_seg(2, 1)), (4, q_seg(2, 0)),
            (6, q_seg(2, 1))],
        3: [(0, k_seg(2, 2)), (2, k_seg(2, 3)), (4, k_seg(3, 0)),
            (6, k_seg(3, 1))],
        4: [(0, k_seg(3, 2)), (2, k_seg(3, 3)), (4, q_seg(3, 0)),
            (6, q_seg(3, 1))],
        5: [(0, q_seg(0, 2)), (2, q_seg(0, 3))],
        6: [(0, q_seg(1, 2)), (2, q_seg(1, 3))],
        7: [(0, q_seg(2, 2)), (2, q_seg(2, 3))],
        8: [(0, q_seg(3, 2)), (2, q_seg(3, 3))],
        9: [],
        10: [(1, outproj(0)), (5, outproj(1))],
        11: [(1, outproj(2)), (5, outproj(3))],
        12: [(1, outproj(4)), (5, outproj(5))],
        13: [(1, outproj(6)), (5, outproj(7))],
        14: [], 15: [],
    }

    # ---- attention: 16 units (q-half, head-pair, head), software-
    # pipelined one kt-pair ahead: scores for step t+1 are emitted right
    # after the exps of step t, so the next exp never waits for attnV.
    todos = {ui: list(fillers[ui]) for ui in range(len(units))}
    for ti, (ui, pi) in enumerate(tasks):
        qh, jt, hh = units[ui]
        q0 = qh * 1024
        r0 = hh * 64
        h = 2 * jt + hh
        if pi == 0:
            avps[ui] = ps_av(f"av{ui}")
        avp = avps[ui]
        sTs = sts.pop((ui, pi))
        et2 = epool.tile([128, 2048], BF16, tag="e", name="et", bufs=3)
        ets = [et2[:, 0:1024], et2[:, 1024:2048]]
        a0, a1 = sTs[0][:], sTs[1][:]
        if (a1.offset - a0.offset) == a0.ap[1][0] * 1024:
            # the two score buffers are adjacent in PSUM: one N=2048
            # activation covers both k-tiles (saves the ~290ns ACT ramp)
            merged = bass.AP(tensor=a0.tensor, offset=a0.offset,
                             ap=[list(a0.ap[0]), [a0.ap[1][0], 2048]])
            nc.scalar.activation(et2[:], merged, EXP, bias=ebias)
        else:
            for par in range(2):
                nc.scalar.activation(ets[par], sTs[par][:], EXP, bias=ebias)
        # prefetch next unit's qdup mid-unit so its scores never wait
        if pi == 3 and ui + 1 < len(units):
            nqh, njt, nhh = units[ui + 1]
            qdups[ui + 1] = qdup_for(njt, nqh, nhh)
        # scores one step ahead (possibly crossing into the next unit)
        if ti + 1 < len(tasks):
            nui, npi = tasks[ti + 1]
            sts[(nui, npi)] = emit_scores(nui, npi)
        for par in range(2):
            kt = 2 * pi + par
            for qbh in range(2):
                nc.tensor.matmul(
                    avp[0:DK + 1, qbh * 512:(qbh + 1) * 512],
                    lhsT=v_t[kt][:, h, 0:DK + 1],
                    rhs=ets[par][:, qbh * 512:(qbh + 1) * 512],
                    start=(kt == 0),
                    stop=(kt == NKT - 1),
                )
        todo = todos[ui]
        if pi < NKT // 2 - 1:
            while todo and todo[0][0] <= pi:
                todo.pop(0)[1]()
            continue
        # ---- end of unit: epilogue first (its avsb copy releases avp
        # for the next unit), then leftover fillers ----
        avsb = smalls.tile([128, 1024], F32, tag="avsb", name="avsb", bufs=1)
        nc.vector.tensor_copy(out=avsb[DK:DK + 1, :], in_=avp[DK:DK + 1, :])
        rsb = smalls.tile([128, 16], F32, tag="rsb", name="rsb", bufs=2)
        nc.gpsimd.dma_start(out=rsb[0:64, :], in_=avsb[DK:DK + 1, :])
        nc.vector.tensor_copy(out=avsb[0:DK, :], in_=avp[0:DK, :])
        rc2 = smalls.tile([128, 16], F32, tag="rc2", name="rc2", bufs=2)
        nc.vector.reciprocal(rc2[0:64, :], rsb[0:64, :])
        rdram = dramp.tile([1024], F32, tag="rdram", name="rdram", bufs=2)
        nc.gpsimd.dma_start(out=rdram.rearrange("(p a) -> p a", p=64),
                            in_=rc2[0:64, :])
        rb = smalls.tile([128, 1024], F32, tag="rb", name="rb", bufs=1)
        nc.gpsimd.dma_start(out=rb[0:DK, :],
                            in_=_bcast_rows(rdram[None, :], DK))
        nc.vector.scalar_tensor_tensor(
            out=oc_t[jt][r0:r0 + 64, q0:q0 + 1024],
            in0=avsb[0:DK, :], scalar=1.0, in1=rb[0:DK, :],
            op0=MULT, op1=MULT)
        while todo:
            todo.pop(0)[1]()

    # ---- remaining out-projection (qh1 rows) ----
    for qt in range(8, NKT):
        outproj(qt)()


_CACHE = {}


def _build():
    if "nc" in _CACHE:
        return _CACHE["nc"]
    from contextlib import ExitStack

    nc = bacc.Bacc("TRN2", target_bir_lowering=False, debug=False,
                   num_devices=NCORES)
    with tile.TileContext(nc) as tc:
        with ExitStack() as ctx:
            _emit(nc, tc, ctx)
    nc.compile()
    _CACHE["nc"] = nc
    return nc


def _stage_segs(xT, nseg, tseg):
    """[D, S] (d = n*128+p) -> [nseg, 128p, 8n, tseg] contiguous."""
    A = xT.reshape(NDT, 128, nseg, tseg)
    return np.ascontiguousarray(A.transpose(2, 1, 0, 3))


def _stage_w(W):
    """[D, J] (d = n*128+p) -> [128p, n, J] contiguous."""
    A = W.reshape(-1, 128, W.shape[1])
    return np.ascontiguousarray(A.transpose(1, 0, 2))


def make_in_maps(query, key, value, Wq, bq, Wk, bk, Wv, bv, Wo, bo):
    import ml_dtypes
    bf = ml_dtypes.bfloat16
    f32 = np.float32
    query = np.asarray(query, f32)
    key = np.asarray(key, f32)
    value = np.asarray(value, f32)
    Wq, Wk, Wv, Wo = (np.asarray(a, f32) for a in (Wq, Wk, Wv, Wo))
    bq, bk, bv = (np.asarray(a, f32) for a in (bq, bk, bv))
    scale = f32(1.0 / np.sqrt(DK))
    xT = {}
    for b in range(B):
        xT[b] = (
            _stage_segs(query[b].T.astype(bf), 4, 512),
            _stage_segs(key[b].T.astype(bf), 4, 512),
            _stage_segs(value[b].T.astype(bf), 8, 256),
        )
    in_maps = []
    for c in range(NCORES):
        b, hh = divmod(c, 2)
        js = slice(hh * DH, (hh + 1) * DH)
        xqT, xkT, xvT = xT[b]
        in_maps.append({
            "xq": xqT,
            "xk": xkT,
            "xv": xvT,
            "wq": _stage_w((Wq[:, js] * scale).astype(bf)),
            "bq": np.ascontiguousarray((bq[js] * scale).reshape(NJT, 128).T),
            "wk": _stage_w(Wk[:, js].astype(bf)),
            "bk": np.ascontiguousarray(bk[js].reshape(NJT, 128).T),
            "wv": _stage_w(Wv[:, js].astype(bf)),
            "bv": np.ascontiguousarray(np.broadcast_to(bv[js], (128, DH))),
            "wo": _stage_w(Wo[js, :].astype(bf)),
        })
    return in_maps


LAST_RESULTS = None


def kernel(query, key, value, Wq, bq, Wk, bk, Wv, bv, Wo, bo):
    global LAST_RESULTS
    import os
    from concourse.bass_utils import run_bass_kernel_spmd

    nc = _build()
    in_maps = make_in_maps(query, key, value, Wq, bq, Wk, bk, Wv, bv, Wo, bo)
    trace = bool(int(os.environ.get("KERNEL_TRACE", "0")))
    res = run_bass_kernel_spmd(nc, in_maps, list(range(NCORES)), trace=trace)
    LAST_RESULTS = res
    bo32 = np.asarray(bo, dtype=np.float32)
    out = np.empty((B, S, D), dtype=np.float32)
    for b in range(B):
        out[b] = (res.results[2 * b]["y"].astype(np.float32)
                  + res.results[2 * b + 1]["y"].astype(np.float32) + bo32)
    return out


# revision 84
# speedup vs baseline: 1.1646x; 1.0048x over previous
"""Trainium2 8-core kernel for MemoryEfficientAttention.

Full multi-head attention layer: Q/K/V projections + exact-softmax attention
+ output projection for [B=4, S=2048, D=1024], H=16 heads, dk=64.

Sharding: core c handles batch c//2 and head-half c%2 (8 heads = 512 dims).
Each core produces a partial out-projection [2048, 1024] in bf16; the host
sums the two partials per batch (fp32) and adds the output bias.

Optimizations over the original baseline (~503us -> ~417us):
- host stages all inputs/weights in DMA-friendly per-partition-contiguous
  blocks (xq/xk as 4 token segments, xv as 8 chunks, weights/biases
  pre-transposed) so every input DMA uses large descriptors.
- DMA queue plan measured against the HW arbiter: the scalar queue gets
  strict service priority, so it carries the prologue-critical slice
  (wk, xk0, wq, xq0, xq1, wv) in need order; gpsimd (independent SW
  descriptor-gen) carries the small latency-critical SBUF moves (kstg
  scatters, qdup staging, softmax epilogue bounce); sync carries the
  deferred bulk (xk1-3, xq2-3, wo, y).
- attention starts as soon as K(jt0)+Q(jt0,qh0) are projected (~30us);
  V projection and remaining K/Q segs run as deadline-scheduled fillers
  (one ~1.7us item per kt-pair step, popped at the END of the step body).
- the kt-pair loop is software-pipelined one step ahead: scores for
  step t+1 are emitted right after the exps of step t, so the next exp
  never waits on attnV/fillers; scores are par-major so a PSUM-rotation
  WAR can't head-block the independent pair.
- separate tiles per input segment and split kh lo/hi pair-range tiles
  keep the tile framework's whole-tile dependencies precise.
- projection bias-adds on DVE (tensor_scalar, per-partition bias AP);
  ScalarE does only exp (the 256 exps at ~1.15us are the pacing engine).
- outproj(0..7) fill the PE-idle late units 10-13; tail is outproj(8..15).

Per-core dataflow (bf16 matmuls, PSUM fp32):
  QT[jt] = Wq[:,jt]^T @ xqT  [128 j, 2048 q]
  KT -> kh pair layout per head [128, 8 kt-pairs, 128]
  V[kt] [128 tok, 8 heads, 64+1] with a ones column (softmax denominator)
  per unit (q-half, head): 16 units, 8 kt-pair steps:
    sT = K_h @ Q_h   [128 k, 1024 q] PSUM (row-split pair via tile_position)
    eT = exp(sT)     ScalarE -> bf16 SBUF
    av += V_aug^T @ eT  [65, 1024] PSUM accumulated over 16 kt
  oc = av[0:64] * (1/av[64])  DVE reciprocal + row-broadcast DMA + DVE mult
  y  = oc^T @ Wo   [2048, 1024] bf16 partial
"""

import numpy as np

import concourse.bass as bass
import concourse.mybir as mybir
import concourse.tile as tile
from concourse import bacc

B, S, D, H, DK = 4, 2048, 1024, 16, 64
NCORES = 8
HPC = H // 2          # heads per core
DH = HPC * DK         # 512 projection dims per core
NJT = 4               # head pairs per core
NDT = D // 128        # 8 d-tiles
NKT = S // 128        # 16 k-tiles
F32 = mybir.dt.float32
BF16 = mybir.dt.bfloat16
U16 = mybir.dt.uint16
EXP = mybir.ActivationFunctionType.Exp
MULT = mybir.AluOpType.mult
ADD = mybir.AluOpType.add

# pair-Schraudolph constants (bf16 bit trick; dormant unless USE_SCH)
SCH_A = 128.0 / np.log(2.0)
SCH_D = np.log(2.0) / 4.0
SCH_C = 0.3
SCH_B1 = 16256.0 - SCH_C - SCH_A * SCH_D
SCH_B2 = 16256.0 - SCH_C + SCH_A * SCH_D
EXP_BIAS = 0.743598

USE_SCH = False


def _sch_tile(hh, kt):
    return USE_SCH and kt % 8 == 3


def _bcast_rows(ap_row, nrows):
    """AP that reads one partition row `nrows` times (partition step 0)."""
    return bass.AP(
        tensor=ap_row.tensor,
        offset=ap_row.offset,
        ap=[[0, nrows]] + [list(x) for x in ap_row.ap[1:]],
    )


def _emit(nc, tc, ctx):
    xq = nc.dram_tensor("xq", [4, 128, NDT, 512], BF16,
                        kind="ExternalInput").ap()
    xk = nc.dram_tensor("xk", [4, 128, NDT, 512], BF16,
                        kind="ExternalInput").ap()
    xv = nc.dram_tensor("xv", [8, 128, NDT, 256], BF16,
                        kind="ExternalInput").ap()
    wq = nc.dram_tensor("wq", [128, NDT, DH], BF16, kind="ExternalInput").ap()
    wk = nc.dram_tensor("wk", [128, NDT, DH], BF16, kind="ExternalInput").ap()
    wv = nc.dram_tensor("wv", [128, NDT, DH], BF16, kind="ExternalInput").ap()
    wo = nc.dram_tensor("wo", [128, NJT, D], BF16, kind="ExternalInput").ap()
    bq = nc.dram_tensor("bq", [128, NJT], F32, kind="ExternalInput").ap()
    bk = nc.dram_tensor("bk", [128, NJT], F32, kind="ExternalInput").ap()
    bv = nc.dram_tensor("bv", [128, DH], F32, kind="ExternalInput").ap()
    y = nc.dram_tensor("y", [S, D], BF16, kind="ExternalOutput").ap()

    consts = ctx.enter_context(tc.tile_pool(name="consts", bufs=1))
    wpool = ctx.enter_context(tc.tile_pool(name="weights", bufs=2))
    xvpool = ctx.enter_context(tc.tile_pool(name="xvchunks", bufs=3))
    xqkpool = ctx.enter_context(tc.tile_pool(name="xqk", bufs=1))
    qkpool = ctx.enter_context(tc.tile_pool(name="qk", bufs=1))
    vpool = ctx.enter_context(tc.tile_pool(name="vps", bufs=1))
    ocpool = ctx.enter_context(tc.tile_pool(name="ocp", bufs=1))
    epool = ctx.enter_context(tc.tile_pool(name="expt", bufs=3))
    smalls = ctx.enter_context(tc.tile_pool(name="smalls", bufs=2))
    ypool = ctx.enter_context(tc.tile_pool(name="ystage", bufs=2))
    dramp = ctx.enter_context(tc.tile_pool(name="drams", bufs=1, space="DRAM"))
    psum = ctx.enter_context(tc.tile_pool(name="psum", bufs=1, space="PSUM"))
    if USE_SCH:
        upool = ctx.enter_context(tc.tile_pool(name="u16", bufs=2))

    # PSUM: tag s = [128,1024] x3 (6 banks; scores rotate + proj/outproj
    # fillers borrow), av = [128,1024] (2 banks)
    def ps_s(name):
        return psum.tile([128, 1024], F32, tag="s", name=name, bufs=3)

    def ps_av(name):
        return psum.tile([128, 1024], F32, tag="av", name=name, bufs=1)

    def ps_aux(name):
        return psum.tile([128, 1024], F32, tag="s", name=name, bufs=3)[:, 0:512]

    ebias = consts.tile([128, 1], F32)
    nc.vector.memset(ebias, EXP_BIAS if USE_SCH else 0.0)

    qt_t = [qkpool.tile([128, S], BF16, tag=f"q{jt}", name=f"qT{jt}")
            for jt in range(NJT)]
    # K pair layout, per head: [128, 4 kt-pairs, 128]; rows 0:64 = even
    # k-tile, rows 64:128 = odd k-tile (row-split pair scores). Split into
    # lo (pi 0-3) / hi (pi 4-7) tiles so a k_seg filler writing the hi half
    # doesn't serialize the current unit's lo-half scores (whole-tile deps).
    kh_lo = [qkpool.tile([128, NKT // 4, 128], BF16, tag=f"kl{h}",
                         name=f"khl{h}") for h in range(HPC)]
    kh_hi = [qkpool.tile([128, NKT // 4, 128], BF16, tag=f"kh{h}",
                         name=f"khh{h}") for h in range(HPC)]
    v_t = [vpool.tile([128, HPC, DK + 2], BF16, tag=f"v{kt}", name=f"v{kt}")
           for kt in range(NKT)]
    oc_t = [ocpool.tile([128, S], BF16, tag=f"oc{jt}", name=f"oc{jt}")
            for jt in range(NJT)]

    # ---- critical-path DMAs. The HW descriptor-gen serves the scalar
    # queue with strict priority over sync (gpsimd runs its own SW DGE),
    # so: critical slice on scalar+gpsimd in need order, deferred bulk on
    # sync, and all small latency-critical SBUF moves (kstg/qdup/y/
    # epilogue) on gpsimd whose engine is otherwise idle.
    xk_sb = [xqkpool.tile([128, NDT, 512], BF16, tag=f"xk{s}",
                          name=f"xk_sb{s}") for s in range(4)]
    xq_sb = [xqkpool.tile([128, NDT, 512], BF16, tag=f"xq{s}",
                          name=f"xq_sb{s}") for s in range(4)]

    wk_sb = wpool.tile([128, NDT, DH], BF16, tag="wqk", name="w_k", bufs=1)
    nc.scalar.dma_start(out=wk_sb, in_=wk)
    nc.scalar.dma_start(out=xk_sb[0], in_=xk[0])
    wq_sb = wpool.tile([128, NDT, DH], BF16, tag="wqk2", name="w_q", bufs=1)
    nc.scalar.dma_start(out=wq_sb, in_=wq)
    nc.scalar.dma_start(out=xq_sb[0], in_=xq[0])
    nc.scalar.dma_start(out=xq_sb[1], in_=xq[1])
    wv_sb = wpool.tile([128, NDT, DH], BF16, tag="w", name="w_v", bufs=1)
    nc.scalar.dma_start(out=wv_sb, in_=wv)

    bq_sb = consts.tile([128, NJT], F32)
    nc.sync.dma_start(out=bq_sb, in_=bq)
    bk_sb = consts.tile([128, NJT], F32)
    nc.sync.dma_start(out=bk_sb, in_=bk)
    bv_row = consts.tile([128, DH], F32)
    nc.sync.dma_start(out=bv_row, in_=bv)
    nc.sync.dma_start(out=xk_sb[1], in_=xk[1])

    # ---- V projection per 256-token chunk (kt pair) ----
    def v_chunk(c):
        def emit():
            xc = xvpool.tile([128, NDT, 256], BF16, tag="xv", name=f"xv{c}",
                             bufs=2)
            nc.gpsimd.dma_start(out=xc, in_=xv[c])
            for ktl in range(2):
                kt = 2 * c + ktl
                pv = ps_aux(f"pv{kt}")
                for dt in range(NDT):
                    nc.tensor.matmul(
                        pv[:],
                        lhsT=xc[:, dt, ktl * 128:(ktl + 1) * 128],
                        rhs=wv_sb[:, dt, :],
                        start=(dt == 0),
                        stop=(dt == NDT - 1),
                    )
                vt = v_t[kt]
                nc.vector.memset(vt[:, :, DK:DK + 1], 1.0)
                nc.vector.tensor_add(
                    out=vt[:, :, 0:DK],
                    in0=pv.rearrange("p (h d) -> p h d", h=HPC),
                    in1=bv_row.rearrange("p (h d) -> p h d", h=HPC),
                )
        return emit

    # ---- K projection per (jt, seg): 512 k-tokens -> kh pair layout ----
    def k_seg(jt, seg):
        def emit():
            pq = ps_aux(f"pk{jt}{seg}")
            for dt in range(NDT):
                nc.tensor.matmul(
                    pq[:],
                    lhsT=wk_sb[:, dt, jt * 128:(jt + 1) * 128],
                    rhs=xk_sb[seg][:, dt, :],
                    start=(dt == 0),
                    stop=(dt == NDT - 1),
                )
            stage = smalls.tile([128, 4, 128], BF16, tag="kstg",
                                name="kstg", bufs=2)
            nc.vector.tensor_scalar(
                out=stage[:], in0=pq.rearrange("p (a b) -> p a b", a=4),
                scalar1=bk_sb[:, jt:jt + 1], scalar2=None, op0=ADD)
            kh = kh_lo if seg < 2 else kh_hi
            sg = seg % 2
            for hh in range(2):
                h = 2 * jt + hh
                r0 = hh * 64
                for par in range(2):   # kt parity within the seg
                    nc.gpsimd.dma_start(
                        out=kh[h][par * 64:par * 64 + 64,
                                  2 * sg:2 * sg + 2, :],
                        in_=stage[r0:r0 + 64, par::2, :],
                    )
        return emit

    def k_segs(jt, segs):
        def emit():
            for seg in segs:
                k_seg(jt, seg)()
        return emit

    # ---- Q projection per (jt, seg) ----
    def q_seg(jt, seg):
        def emit():
            pq = ps_aux(f"pq{jt}{seg}")
            for dt in range(NDT):
                nc.tensor.matmul(
                    pq[:],
                    lhsT=wq_sb[:, dt, jt * 128:(jt + 1) * 128],
                    rhs=xq_sb[seg][:, dt, :],
                    start=(dt == 0),
                    stop=(dt == NDT - 1),
                )
            nc.vector.tensor_scalar(
                out=qt_t[jt][:, seg * 512:(seg + 1) * 512], in0=pq[:],
                scalar1=bq_sb[:, jt:jt + 1], scalar2=None, op0=ADD)
        return emit

    def q_segs(jt, segs):
        def emit():
            for seg in segs:
                q_seg(jt, seg)()
        return emit

    # ---- out-projection closure (one q-row-tile) ----
    def outproj(qt):
        def emit():
            py = psum.tile([128, 1024], F32, tag="s", name=f"py{qt}", bufs=3)
            for nb in range(2):
                for jt in range(NJT):
                    nc.tensor.matmul(
                        py[:, nb * 512:(nb + 1) * 512],
                        lhsT=oc_t[jt][:, qt * 128:(qt + 1) * 128],
                        rhs=wo_sb[:, jt, nb * 512:(nb + 1) * 512],
                        start=(jt == 0),
                        stop=(jt == NJT - 1),
                    )
            ys = ypool.tile([128, 1024], BF16, tag="y", name="ys", bufs=2)
            nc.vector.tensor_copy(out=ys[:], in_=py[:])
            nc.sync.dma_start(out=y[qt * 128:(qt + 1) * 128, :], in_=ys[:])
        return emit

    # ---- qdup staging: duplicate a head's Q rows into both halves ----
    def qdup_for(jt, qh, hh):
        q0 = qh * 1024
        r0 = hh * 64
        qd = smalls.tile([128, 1024], BF16, tag="qh", name="qdup", bufs=2)
        for half in range(2):
            nc.gpsimd.dma_start(out=qd[half * 64:half * 64 + 64, :],
                                in_=qt_t[jt][r0:r0 + 64, q0:q0 + 1024])
        return qd

    units = [(qh, jt, hh) for qh in range(2) for jt in range(NJT)
             for hh in range(2)]
    tasks = [(ui, pi) for ui in range(len(units)) for pi in range(NKT // 2)]
    qdups = {}
    avps = {}
    sts = {}

    def emit_scores(ui, pi):
        _, jt_, hh_ = units[ui]
        h_ = 2 * jt_ + hh_
        sTs = [ps_s(f"sT{ui}_{2 * pi + p}") for p in range(2)]
        kh = kh_lo[h_] if pi < 4 else kh_hi[h_]
        qdup = qdups[ui]
        # alternate the two row-split tiles so their streams overlap in
        # the PE array (same-tile matmuls back-to-back would serialize
        # and head-of-line-block the independent tile behind them).
        for qbh in range(2):
            for par in range(2):
                nc.tensor.matmul(
                    sTs[par][:, qbh * 512:(qbh + 1) * 512],
                    lhsT=kh[par * 64:par * 64 + 64, pi % 4, :],
                    rhs=qdup[par * 64:par * 64 + 64,
                             qbh * 512:(qbh + 1) * 512],
                    start=True,
                    stop=True,
                    tile_position=(par * 64, 0),
                )
        return sTs

    # ---- prologue: minimum work before unit 0's first scores; the
    # first scores are emitted BEFORE the V chunks so the first exp
    # doesn't sit behind them in the PE program. ----
    k_seg(0, 0)()
    k_seg(0, 1)()
    q_seg(0, 0)()
    q_seg(0, 1)()
    qdups[0] = qdup_for(0, 0, 0)
    sts[(0, 0)] = emit_scores(0, 0)
    v_chunk(0)()
    v_chunk(1)()
    # deferred input DMAs (not needed for the first unit's early steps)
    nc.sync.dma_start(out=xk_sb[2], in_=xk[2])
    nc.sync.dma_start(out=xk_sb[3], in_=xk[3])
    nc.sync.dma_start(out=xq_sb[2], in_=xq[2])
    nc.sync.dma_start(out=xq_sb[3], in_=xq[3])
    wo_sb = wpool.tile([128, NJT, D], BF16, tag="wo", name="w_o", bufs=1)
    nc.sync.dma_start(out=wo_sb, in_=wo)

    # filler schedule: per unit, list of (pi, closure); popped at the END
    # of the kt-pair body whose pi matches (so they never delay scores).
    fillers = {
        0: [(0, v_chunk(2)), (0, k_seg(0, 2)), (1, v_chunk(3)),
            (2, v_chunk(4)), (2, k_seg(0, 3)), (3, v_chunk(5)),
            (4, v_chunk(6)), (5, v_chunk(7))],
        1: [(0, k_seg(1, 0)), (1, k_seg(1, 1)), (2, q_seg(1, 0)),
            (3, q_seg(1, 1)), (4, k_seg(1, 2)), (5, k_seg(1, 3))],
        2: [(0, k_seg(2, 0)), (2, k_seg(2, 1)), (4, q_seg(2, 0)),
            (6, q_seg(2, 1))],
        3: [(0, k_seg(2, 2)), (2, k_seg(2, 3)), (4, k_seg(3, 0)),
            (6, k_seg(3, 1))],
        4: [(0, k_seg(3, 2)), (2, k_seg(3, 3)), (4, q_seg(3, 0)),
            (6, q_seg(3, 1))],
        5: [(0, q_seg(0, 2)), (2, q_seg(0, 3))],
        6: [(0, q_seg(1, 2)), (2, q_seg(1, 3))],
        7: [(0, q_seg(2, 2)), (2, q_seg(2, 3))],
        8: [(0, q_seg(3, 2)), (2, q_seg(3, 3))],
        9: [],
        10: [(1, outproj(0)), (5, outproj(1))],
        11: [(1, outproj(2)), (5, outproj(3))],
        12: [(1, outproj(4)), (5, outproj(5))],
        13: [(1, outproj(6)), (5, outproj(7))],
        14: [], 15: [],
    }

    # ---- attention: 16 units (q-half, head-pair, head), software-
    # pipelined one kt-pair ahead: scores for step t+1 are emitted right
    # after the exps of step t, so the next exp never waits for attnV.
    todos = {ui: list(fillers[ui]) for ui in range(len(units))}
    for ti, (ui, pi) in enumerate(tasks):
        qh, jt, hh = units[ui]
        q0 = qh * 1024
        r0 = hh * 64
        h = 2 * jt + hh
        if pi == 0:
            avps[ui] = ps_av(f"av{ui}")
        avp = avps[ui]
        sTs = sts.pop((ui, pi))
        et2 = epool.tile([128, 2048], BF16, tag="e", name="et", bufs=3)
        ets = [et2[:, 0:1024], et2[:, 1024:2048]]
        a0, a1 = sTs[0][:], sTs[1][:]
        if (a1.offset - a0.offset) == a0.ap[1][0] * 1024:
            # the two score buffers are adjacent in PSUM: one N=2048
            # activation covers both k-tiles (saves the ~290ns ACT ramp)
            merged = bass.AP(tensor=a0.tensor, offset=a0.offset,
                             ap=[list(a0.ap[0]), [a0.ap[1][0], 2048]])
            nc.scalar.activation(et2[:], merged, EXP, bias=ebias)
        else:
            for par in range(2):
                nc.scalar.activation(ets[par], sTs[par][:], EXP, bias=ebias)
        # prefetch next unit's qdup mid-unit so its scores never wait
        if pi == 3 and ui + 1 < len(units):
            nqh, njt, nhh = units[ui + 1]
            qdups[ui + 1] = qdup_for(njt, nqh, nhh)
        # scores one step ahead (possibly crossing into the next unit)
        if ti + 1 < len(tasks):
            nui, npi = tasks[ti + 1]
            sts[(nui, npi)] = emit_scores(nui, npi)
        for par in range(2):
            kt = 2 * pi + par
            for qbh in range(2):
                nc.tensor.matmul(
                    avp[0:DK + 1, qbh * 512:(qbh + 1) * 512],
                    lhsT=v_t[kt][:, h, 0:DK + 1],
                    rhs=ets[par][:, qbh * 512:(qbh + 1) * 512],
                    start=(kt == 0),
                    stop=(kt == NKT - 1),
                )
        todo = todos[ui]
        if pi < NKT // 2 - 1:
            while todo and todo[0][0] <= pi:
                todo.pop(0)[1]()
            continue
        # ---- end of unit: epilogue first (its avsb copy releases avp
        # for the next unit), then leftover fillers ----
        avsb = smalls.tile([128, 1024], F32, tag="avsb", name="avsb", bufs=1)
        nc.vector.tensor_copy(out=avsb[DK:DK + 1, :], in_=avp[DK:DK + 1, :])
        rsb = smalls.tile([128, 16], F32, tag="rsb", name="rsb", bufs=2)
        nc.gpsimd.dma_start(out=rsb[0:64, :], in_=avsb[DK:DK + 1, :])
        nc.vector.tensor_copy(out=avsb[0:DK, :], in_=avp[0:DK, :])
        rc2 = smalls.tile([128, 16], F32, tag="rc2", name="rc2", bufs=2)
        nc.vector.reciprocal(rc2[0:64, :], rsb[0:64, :])
        rdram = dramp.tile([1024], F32, tag="rdram", name="rdram", bufs=2)
        nc.gpsimd.dma_start(out=rdram.rearrange("(p a) -> p a", p=64),
                            in_=rc2[0:64, :])
        rb = smalls.tile([128, 1024], F32, tag="rb", name="rb", bufs=1)
        nc.gpsimd.dma_start(out=rb[0:DK, :],
                            in_=_bcast_rows(rdram[None, :], DK))
        nc.vector.scalar_tensor_tensor(
            out=oc_t[jt][r0:r0 + 64, q0:q0 + 1024],
            in0=avsb[0:DK, :], scalar=1.0, in1=rb[0:DK, :],
            op0=MULT, op1=MULT)
        while todo:
            todo.pop(0)[1]()

    # ---- remaining out-projection (qh1 rows) ----
    for qt in range(8, NKT):
        outproj(qt)()


_CACHE = {}


def _build():
    if "nc" in _CACHE:
        return _CACHE["nc"]
    from contextlib import ExitStack

    nc = bacc.Bacc("TRN2", target_bir_lowering=False, debug=False,
                   num_devices=NCORES)
    with tile.TileContext(nc) as tc:
        with ExitStack() as ctx:
            _emit(nc, tc, ctx)
    nc.compile()
    _CACHE["nc"] = nc
    return nc


def _stage_segs(xT, nseg, tseg):
    """[D, S] (d = n*128+p) -> [nseg, 128p, 8n, tseg] contiguous."""
    A = xT.reshape(NDT, 128, nseg, tseg)
    return np.ascontiguousarray(A.transpose(2, 1, 0, 3))


def _stage_w(W):
    """[D, J] (d = n*128+p) -> [128p, n, J] contiguous."""
    A = W.reshape(-1, 128, W.shape[1])
    return np.ascontiguousarray(A.transpose(1, 0, 2))


def make_in_maps(query, key, value, Wq, bq, Wk, bk, Wv, bv, Wo, bo):
    import ml_dtypes
    bf = ml_dtypes.bfloat16
    f32 = np.float32
    query = np.asarray(query, f32)
    key = np.asarray(key, f32)
    value = np.asarray(value, f32)
    Wq, Wk, Wv, Wo = (np.asarray(a, f32) for a in (Wq, Wk, Wv, Wo))
    bq, bk, bv = (np.asarray(a, f32) for a in (bq, bk, bv))
    scale = f32(1.0 / np.sqrt(DK))
    xT = {}
    for b in range(B):
        xT[b] = (
            _stage_segs(query[b].T.astype(bf), 4, 512),
            _stage_segs(key[b].T.astype(bf), 4, 512),
            _stage_segs(value[b].T.astype(bf), 8, 256),
        )
    in_maps = []
    for c in range(NCORES):
        b, hh = divmod(c, 2)
        js = slice(hh * DH, (hh + 1) * DH)
        xqT, xkT, xvT = xT[b]
        in_maps.append({
            "xq": xqT,
            "xk": xkT,
            "xv": xvT,
            "wq": _stage_w((Wq[:, js] * scale).astype(bf)),
            "bq": np.ascontiguousarray((bq[js] * scale).reshape(NJT, 128).T),
            "wk": _stage_w(Wk[:, js].astype(bf)),
            "bk": np.ascontiguousarray(bk[js].reshape(NJT, 128).T),
            "wv": _stage_w(Wv[:, js].astype(bf)),
            "bv": np.ascontiguousarray(np.broadcast_to(bv[js], (128, DH))),
            "wo": _stage_w(Wo[js, :].astype(bf)),
        })
    return in_maps


LAST_RESULTS = None


def kernel(query, key, value, Wq, bq, Wk, bk, Wv, bv, Wo, bo):
    global LAST_RESULTS
    import os
    from concourse.bass_utils import run_bass_kernel_spmd

    nc = _build()
    in_maps = make_in_maps(query, key, value, Wq, bq, Wk, bk, Wv, bv, Wo, bo)
    trace = bool(int(os.environ.get("KERNEL_TRACE", "0")))
    res = run_bass_kernel_spmd(nc, in_maps, list(range(NCORES)), trace=trace)
    LAST_RESULTS = res
    bo32 = np.asarray(bo, dtype=np.float32)
    out = np.empty((B, S, D), dtype=np.float32)
    for b in range(B):
        out[b] = (res.results[2 * b]["y"].astype(np.float32)
                  + res.results[2 * b + 1]["y"].astype(np.float32) + bo32)
    return out


# revision 86
# speedup vs baseline: 1.1786x; 1.0120x over previous
"""Trainium2 8-core kernel for MemoryEfficientAttention.

Full multi-head attention layer: Q/K/V projections + exact-softmax attention
+ output projection for [B=4, S=2048, D=1024], H=16 heads, dk=64.

Sharding: core c handles batch c//2 and head-half c%2 (8 heads = 512 dims).
Each core produces a partial out-projection [2048, 1024] in bf16; the host
sums the two partials per batch (fp32) and adds the output bias.

Optimizations over the original baseline (~503us -> ~417us):
- host stages all inputs/weights in DMA-friendly per-partition-contiguous
  blocks (xq/xk as 4 token segments, xv as 8 chunks, weights/biases
  pre-transposed) so every input DMA uses large descriptors.
- DMA queue plan measured against the HW arbiter: the scalar queue gets
  strict service priority, so it carries the prologue-critical slice
  (wk, xk0, wq, xq0, xq1, wv) in need order; gpsimd (independent SW
  descriptor-gen) carries the small latency-critical SBUF moves (kstg
  scatters, qdup staging, softmax epilogue bounce); sync carries the
  deferred bulk (xk1-3, xq2-3, wo, y).
- attention starts as soon as K(jt0)+Q(jt0,qh0) are projected (~30us);
  V projection and remaining K/Q segs run as deadline-scheduled fillers
  (one ~1.7us item per kt-pair step, popped at the END of the step body).
- the kt-pair loop is software-pipelined one step ahead: scores for
  step t+1 are emitted right after the exps of step t, so the next exp
  never waits on attnV/fillers; scores are par-major so a PSUM-rotation
  WAR can't head-block the independent pair.
- separate tiles per input segment and split kh lo/hi pair-range tiles
  keep the tile framework's whole-tile dependencies precise.
- projection bias-adds on DVE (tensor_scalar, per-partition bias AP);
  ScalarE does only exp (the 256 exps at ~1.15us are the pacing engine).
- outproj(0..7) fill the PE-idle late units 10-13; tail is outproj(8..15).

Per-core dataflow (bf16 matmuls, PSUM fp32):
  QT[jt] = Wq[:,jt]^T @ xqT  [128 j, 2048 q]
  KT -> kh pair layout per head [128, 8 kt-pairs, 128]
  V[kt] [128 tok, 8 heads, 64+1] with a ones column (softmax denominator)
  per unit (q-half, head): 16 units, 8 kt-pair steps:
    sT = K_h @ Q_h   [128 k, 1024 q] PSUM (row-split pair via tile_position)
    eT = exp(sT)     ScalarE -> bf16 SBUF
    av += V_aug^T @ eT  [65, 1024] PSUM accumulated over 16 kt
  oc = av[0:64] * (1/av[64])  DVE reciprocal + row-broadcast DMA + DVE mult
  y  = oc^T @ Wo   [2048, 1024] bf16 partial
"""

import numpy as np

import concourse.bass as bass
import concourse.mybir as mybir
import concourse.tile as tile
from concourse import bacc

B, S, D, H, DK = 4, 2048, 1024, 16, 64
NCORES = 8
HPC = H // 2          # heads per core
DH = HPC * DK         # 512 projection dims per core
NJT = 4               # head pairs per core
NDT = D // 128        # 8 d-tiles
NKT = S // 128        # 16 k-tiles
F32 = mybir.dt.float32
BF16 = mybir.dt.bfloat16
U16 = mybir.dt.uint16
EXP = mybir.ActivationFunctionType.Exp
MULT = mybir.AluOpType.mult
ADD = mybir.AluOpType.add

# pair-Schraudolph constants (bf16 bit trick; dormant unless USE_SCH)
SCH_A = 128.0 / np.log(2.0)
SCH_D = np.log(2.0) / 4.0
SCH_C = 0.3
SCH_B1 = 16256.0 - SCH_C - SCH_A * SCH_D
SCH_B2 = 16256.0 - SCH_C + SCH_A * SCH_D
EXP_BIAS = 0.743598

USE_SCH = False


def _sch_tile(hh, kt):
    return USE_SCH and kt % 8 == 3


def _bcast_rows(ap_row, nrows):
    """AP that reads one partition row `nrows` times (partition step 0)."""
    return bass.AP(
        tensor=ap_row.tensor,
        offset=ap_row.offset,
        ap=[[0, nrows]] + [list(x) for x in ap_row.ap[1:]],
    )


def _emit(nc, tc, ctx):
    xq = nc.dram_tensor("xq", [4, 128, NDT, 512], BF16,
                        kind="ExternalInput").ap()
    xk = nc.dram_tensor("xk", [4, 128, NDT, 512], BF16,
                        kind="ExternalInput").ap()
    xv = nc.dram_tensor("xv", [8, 128, NDT, 256], BF16,
                        kind="ExternalInput").ap()
    wq = nc.dram_tensor("wq", [128, NDT, DH], BF16, kind="ExternalInput").ap()
    wk = nc.dram_tensor("wk", [128, NDT, DH], BF16, kind="ExternalInput").ap()
    wv = nc.dram_tensor("wv", [128, NDT, DH], BF16, kind="ExternalInput").ap()
    wo = nc.dram_tensor("wo", [128, NJT, D], BF16, kind="ExternalInput").ap()
    bq = nc.dram_tensor("bq", [128, NJT], F32, kind="ExternalInput").ap()
    bk = nc.dram_tensor("bk", [128, NJT], F32, kind="ExternalInput").ap()
    bv = nc.dram_tensor("bv", [128, DH], F32, kind="ExternalInput").ap()
    y = nc.dram_tensor("y", [S, D], BF16, kind="ExternalOutput").ap()

    consts = ctx.enter_context(tc.tile_pool(name="consts", bufs=1))
    wpool = ctx.enter_context(tc.tile_pool(name="weights", bufs=2))
    xvpool = ctx.enter_context(tc.tile_pool(name="xvchunks", bufs=3))
    xqkpool = ctx.enter_context(tc.tile_pool(name="xqk", bufs=1))
    qkpool = ctx.enter_context(tc.tile_pool(name="qk", bufs=1))
    vpool = ctx.enter_context(tc.tile_pool(name="vps", bufs=1))
    ocpool = ctx.enter_context(tc.tile_pool(name="ocp", bufs=1))
    epool = ctx.enter_context(tc.tile_pool(name="expt", bufs=3))
    smalls = ctx.enter_context(tc.tile_pool(name="smalls", bufs=2))
    ypool = ctx.enter_context(tc.tile_pool(name="ystage", bufs=2))
    dramp = ctx.enter_context(tc.tile_pool(name="drams", bufs=1, space="DRAM"))
    psum = ctx.enter_context(tc.tile_pool(name="psum", bufs=1, space="PSUM"))
    if USE_SCH:
        upool = ctx.enter_context(tc.tile_pool(name="u16", bufs=2))

    # PSUM: tag s = [128,1024] x3 (6 banks; scores rotate + proj/outproj
    # fillers borrow), av = [128,1024] (2 banks)
    def ps_s(name):
        return psum.tile([128, 1024], F32, tag="s", name=name, bufs=3)

    def ps_av(name):
        return psum.tile([128, 1024], F32, tag="av", name=name, bufs=1)

    def ps_aux(name):
        return psum.tile([128, 1024], F32, tag="s", name=name, bufs=3)[:, 0:512]

    ebias = consts.tile([128, 1], F32)
    nc.vector.memset(ebias, EXP_BIAS if USE_SCH else 0.0)

    qt_t = [qkpool.tile([128, S], BF16, tag=f"q{jt}", name=f"qT{jt}")
            for jt in range(NJT)]
    # K pair layout, per head: [128, 4 kt-pairs, 128]; rows 0:64 = even
    # k-tile, rows 64:128 = odd k-tile (row-split pair scores). Split into
    # lo (pi 0-3) / hi (pi 4-7) tiles so a k_seg filler writing the hi half
    # doesn't serialize the current unit's lo-half scores (whole-tile deps).
    kh_lo = [qkpool.tile([128, NKT // 4, 128], BF16, tag=f"kl{h}",
                         name=f"khl{h}") for h in range(HPC)]
    kh_hi = [qkpool.tile([128, NKT // 4, 128], BF16, tag=f"kh{h}",
                         name=f"khh{h}") for h in range(HPC)]
    v_t = [vpool.tile([128, HPC, DK + 2], BF16, tag=f"v{kt}", name=f"v{kt}")
           for kt in range(NKT)]
    oc_t = [ocpool.tile([128, S], BF16, tag=f"oc{jt}", name=f"oc{jt}")
            for jt in range(NJT)]

    # ---- critical-path DMAs. The HW descriptor-gen serves the scalar
    # queue with strict priority over sync (gpsimd runs its own SW DGE),
    # so: critical slice on scalar+gpsimd in need order, deferred bulk on
    # sync, and all small latency-critical SBUF moves (kstg/qdup/y/
    # epilogue) on gpsimd whose engine is otherwise idle.
    xk_sb = [xqkpool.tile([128, NDT, 512], BF16, tag=f"xk{s}",
                          name=f"xk_sb{s}") for s in range(4)]
    xq_sb = [xqkpool.tile([128, NDT, 512], BF16, tag=f"xq{s}",
                          name=f"xq_sb{s}") for s in range(4)]

    wk_sb = wpool.tile([128, NDT, DH], BF16, tag="wqk", name="w_k", bufs=1)
    nc.scalar.dma_start(out=wk_sb, in_=wk)
    nc.scalar.dma_start(out=xk_sb[0], in_=xk[0])
    wq_sb = wpool.tile([128, NDT, DH], BF16, tag="wqk2", name="w_q", bufs=1)
    nc.scalar.dma_start(out=wq_sb, in_=wq)
    nc.scalar.dma_start(out=xq_sb[0], in_=xq[0])
    nc.scalar.dma_start(out=xq_sb[1], in_=xq[1])
    wv_sb = wpool.tile([128, NDT, DH], BF16, tag="w", name="w_v", bufs=1)
    nc.scalar.dma_start(out=wv_sb, in_=wv)

    bq_sb = consts.tile([128, NJT], F32)
    nc.sync.dma_start(out=bq_sb, in_=bq)
    bk_sb = consts.tile([128, NJT], F32)
    nc.sync.dma_start(out=bk_sb, in_=bk)
    bv_row = consts.tile([128, DH], F32)
    nc.sync.dma_start(out=bv_row, in_=bv)
    nc.sync.dma_start(out=xk_sb[1], in_=xk[1])

    # ---- V projection per 256-token chunk (kt pair) ----
    def v_chunk(c):
        def emit():
            xc = xvpool.tile([128, NDT, 256], BF16, tag="xv", name=f"xv{c}",
                             bufs=2)
            nc.gpsimd.dma_start(out=xc, in_=xv[c])
            for ktl in range(2):
                kt = 2 * c + ktl
                pv = ps_aux(f"pv{kt}")
                for dt in range(NDT):
                    nc.tensor.matmul(
                        pv[:],
                        lhsT=xc[:, dt, ktl * 128:(ktl + 1) * 128],
                        rhs=wv_sb[:, dt, :],
                        start=(dt == 0),
                        stop=(dt == NDT - 1),
                    )
                vt = v_t[kt]
                nc.vector.memset(vt[:, :, DK:DK + 1], 1.0)
                nc.vector.tensor_add(
                    out=vt[:, :, 0:DK],
                    in0=pv.rearrange("p (h d) -> p h d", h=HPC),
                    in1=bv_row.rearrange("p (h d) -> p h d", h=HPC),
                )
        return emit

    # ---- K projection per (jt, seg): 512 k-tokens -> kh pair layout ----
    def k_seg(jt, seg):
        def emit():
            pq = ps_aux(f"pk{jt}{seg}")
            for dt in range(NDT):
                nc.tensor.matmul(
                    pq[:],
                    lhsT=wk_sb[:, dt, jt * 128:(jt + 1) * 128],
                    rhs=xk_sb[seg][:, dt, :],
                    start=(dt == 0),
                    stop=(dt == NDT - 1),
                )
            stage = smalls.tile([128, 4, 128], BF16, tag="kstg",
                                name="kstg", bufs=2)
            nc.vector.tensor_scalar(
                out=stage[:], in0=pq.rearrange("p (a b) -> p a b", a=4),
                scalar1=bk_sb[:, jt:jt + 1], scalar2=None, op0=ADD)
            kh = kh_lo if seg < 2 else kh_hi
            sg = seg % 2
            for hh in range(2):
                h = 2 * jt + hh
                r0 = hh * 64
                for par in range(2):   # kt parity within the seg
                    nc.gpsimd.dma_start(
                        out=kh[h][par * 64:par * 64 + 64,
                                  2 * sg:2 * sg + 2, :],
                        in_=stage[r0:r0 + 64, par::2, :],
                    )
        return emit

    def k_segs(jt, segs):
        def emit():
            for seg in segs:
                k_seg(jt, seg)()
        return emit

    # ---- Q projection per (jt, seg) ----
    def q_seg(jt, seg):
        def emit():
            pq = ps_aux(f"pq{jt}{seg}")
            for dt in range(NDT):
                nc.tensor.matmul(
                    pq[:],
                    lhsT=wq_sb[:, dt, jt * 128:(jt + 1) * 128],
                    rhs=xq_sb[seg][:, dt, :],
                    start=(dt == 0),
                    stop=(dt == NDT - 1),
                )
            nc.vector.tensor_scalar(
                out=qt_t[jt][:, seg * 512:(seg + 1) * 512], in0=pq[:],
                scalar1=bq_sb[:, jt:jt + 1], scalar2=None, op0=ADD)
        return emit

    def q_segs(jt, segs):
        def emit():
            for seg in segs:
                q_seg(jt, seg)()
        return emit

    # ---- out-projection closure (one q-row-tile) ----
    def outproj(qt, tail=False):
        def emit():
            py = psum.tile([128, 1024], F32, tag="s", name=f"py{qt}", bufs=3)
            for nb in range(2):
                for jt in range(NJT):
                    nc.tensor.matmul(
                        py[:, nb * 512:(nb + 1) * 512],
                        lhsT=oc_t[jt][:, qt * 128:(qt + 1) * 128],
                        rhs=wo_sb[:, jt, nb * 512:(nb + 1) * 512],
                        start=(jt == 0),
                        stop=(jt == NJT - 1),
                    )
            ys = ypool.tile([128, 1024], BF16, tag="y", name="ys", bufs=2)
            if tail:
                # post-last-exp, ScalarE is idle and has the faster PSUM
                # port; keep the DVE free for the final epilogue chain
                nc.scalar.copy(out=ys[:], in_=py[:])
            else:
                nc.vector.tensor_copy(out=ys[:], in_=py[:])
            nc.sync.dma_start(out=y[qt * 128:(qt + 1) * 128, :], in_=ys[:])
        return emit

    # ---- qdup staging: duplicate a head's Q rows into both halves ----
    def qdup_for(jt, qh, hh):
        q0 = qh * 1024
        r0 = hh * 64
        qd = smalls.tile([128, 1024], BF16, tag="qh", name="qdup", bufs=2)
        for half in range(2):
            nc.gpsimd.dma_start(out=qd[half * 64:half * 64 + 64, :],
                                in_=qt_t[jt][r0:r0 + 64, q0:q0 + 1024])
        return qd

    units = [(qh, jt, hh) for qh in range(2) for jt in range(NJT)
             for hh in range(2)]
    tasks = [(ui, pi) for ui in range(len(units)) for pi in range(NKT // 2)]
    qdups = {}
    avps = {}
    sts = {}

    def emit_scores(ui, pi):
        _, jt_, hh_ = units[ui]
        h_ = 2 * jt_ + hh_
        sTs = [ps_s(f"sT{ui}_{2 * pi + p}") for p in range(2)]
        kh = kh_lo[h_] if pi < 4 else kh_hi[h_]
        qdup = qdups[ui]
        # alternate the two row-split tiles so their streams overlap in
        # the PE array (same-tile matmuls back-to-back would serialize
        # and head-of-line-block the independent tile behind them).
        for qbh in range(2):
            for par in range(2):
                nc.tensor.matmul(
                    sTs[par][:, qbh * 512:(qbh + 1) * 512],
                    lhsT=kh[par * 64:par * 64 + 64, pi % 4, :],
                    rhs=qdup[par * 64:par * 64 + 64,
                             qbh * 512:(qbh + 1) * 512],
                    start=True,
                    stop=True,
                    tile_position=(par * 64, 0),
                )
        return sTs

    # ---- prologue: minimum work before unit 0's first scores; the
    # first scores are emitted BEFORE the V chunks so the first exp
    # doesn't sit behind them in the PE program. ----
    k_seg(0, 0)()
    k_seg(0, 1)()
    q_seg(0, 0)()
    q_seg(0, 1)()
    qdups[0] = qdup_for(0, 0, 0)
    sts[(0, 0)] = emit_scores(0, 0)
    v_chunk(0)()
    v_chunk(1)()
    # deferred input DMAs (not needed for the first unit's early steps)
    nc.sync.dma_start(out=xk_sb[2], in_=xk[2])
    nc.sync.dma_start(out=xk_sb[3], in_=xk[3])
    nc.sync.dma_start(out=xq_sb[2], in_=xq[2])
    nc.sync.dma_start(out=xq_sb[3], in_=xq[3])
    wo_sb = wpool.tile([128, NJT, D], BF16, tag="wo", name="w_o", bufs=1)
    nc.sync.dma_start(out=wo_sb, in_=wo)

    # filler schedule: per unit, list of (pi, closure); popped at the END
    # of the kt-pair body whose pi matches (so they never delay scores).
    fillers = {
        0: [(0, v_chunk(2)), (0, k_seg(0, 2)), (1, v_chunk(3)),
            (2, v_chunk(4)), (2, k_seg(0, 3)), (3, v_chunk(5)),
            (4, v_chunk(6)), (5, v_chunk(7))],
        1: [(0, k_seg(1, 0)), (1, k_seg(1, 1)), (2, q_seg(1, 0)),
            (3, q_seg(1, 1)), (4, k_seg(1, 2)), (5, k_seg(1, 3))],
        2: [(0, k_seg(2, 0)), (2, k_seg(2, 1)), (4, q_seg(2, 0)),
            (6, q_seg(2, 1))],
        3: [(0, k_seg(2, 2)), (2, k_seg(2, 3)), (4, k_seg(3, 0)),
            (6, k_seg(3, 1))],
        4: [(0, k_seg(3, 2)), (2, k_seg(3, 3)), (4, q_seg(3, 0)),
            (6, q_seg(3, 1))],
        5: [(0, q_seg(0, 2)), (2, q_seg(0, 3))],
        6: [(0, q_seg(1, 2)), (2, q_seg(1, 3))],
        7: [(0, q_seg(2, 2)), (2, q_seg(2, 3))],
        8: [(0, q_seg(3, 2)), (2, q_seg(3, 3))],
        9: [],
        10: [(1, outproj(0)), (5, outproj(1))],
        11: [(1, outproj(2)), (5, outproj(3))],
        12: [(1, outproj(4)), (5, outproj(5))],
        13: [(1, outproj(6)), (5, outproj(7))],
        14: [], 15: [],
    }

    # ---- attention: 16 units (q-half, head-pair, head), software-
    # pipelined one kt-pair ahead: scores for step t+1 are emitted right
    # after the exps of step t, so the next exp never waits for attnV.
    todos = {ui: list(fillers[ui]) for ui in range(len(units))}
    for ti, (ui, pi) in enumerate(tasks):
        qh, jt, hh = units[ui]
        q0 = qh * 1024
        r0 = hh * 64
        h = 2 * jt + hh
        if pi == 0:
            avps[ui] = ps_av(f"av{ui}")
        avp = avps[ui]
        sTs = sts.pop((ui, pi))
        et2 = epool.tile([128, 2048], BF16, tag="e", name="et", bufs=3)
        ets = [et2[:, 0:1024], et2[:, 1024:2048]]
        a0, a1 = sTs[0][:], sTs[1][:]
        if (a1.offset - a0.offset) == a0.ap[1][0] * 1024:
            # the two score buffers are adjacent in PSUM: one N=2048
            # activation covers both k-tiles (saves the ~290ns ACT ramp)
            merged = bass.AP(tensor=a0.tensor, offset=a0.offset,
                             ap=[list(a0.ap[0]), [a0.ap[1][0], 2048]])
            nc.scalar.activation(et2[:], merged, EXP, bias=ebias)
        else:
            for par in range(2):
                nc.scalar.activation(ets[par], sTs[par][:], EXP, bias=ebias)
        # prefetch next unit's qdup mid-unit so its scores never wait
        if pi == 3 and ui + 1 < len(units):
            nqh, njt, nhh = units[ui + 1]
            qdups[ui + 1] = qdup_for(njt, nqh, nhh)
        # scores one step ahead (possibly crossing into the next unit)
        if ti + 1 < len(tasks):
            nui, npi = tasks[ti + 1]
            sts[(nui, npi)] = emit_scores(nui, npi)
        for par in range(2):
            kt = 2 * pi + par
            for qbh in range(2):
                nc.tensor.matmul(
                    avp[0:DK + 1, qbh * 512:(qbh + 1) * 512],
                    lhsT=v_t[kt][:, h, 0:DK + 1],
                    rhs=ets[par][:, qbh * 512:(qbh + 1) * 512],
                    start=(kt == 0),
                    stop=(kt == NKT - 1),
                )
        todo = todos[ui]
        if pi < NKT // 2 - 1:
            while todo and todo[0][0] <= pi:
                todo.pop(0)[1]()
            continue
        # ---- end of unit: epilogue first (its avsb copy releases avp
        # for the next unit), then leftover fillers ----
        avsb = smalls.tile([128, 1024], F32, tag="avsb", name="avsb", bufs=1)
        nc.vector.tensor_copy(out=avsb[DK:DK + 1, :], in_=avp[DK:DK + 1, :])
        rsb = smalls.tile([128, 16], F32, tag="rsb", name="rsb", bufs=2)
        nc.gpsimd.dma_start(out=rsb[0:64, :], in_=avsb[DK:DK + 1, :])
        nc.vector.tensor_copy(out=avsb[0:DK, :], in_=avp[0:DK, :])
        rc2 = smalls.tile([128, 16], F32, tag="rc2", name="rc2", bufs=2)
        nc.vector.reciprocal(rc2[0:64, :], rsb[0:64, :])
        rdram = dramp.tile([1024], F32, tag="rdram", name="rdram", bufs=2)
        nc.gpsimd.dma_start(out=rdram.rearrange("(p a) -> p a", p=64),
                            in_=rc2[0:64, :])
        rb = smalls.tile([128, 1024], F32, tag="rb", name="rb", bufs=1)
        nc.gpsimd.dma_start(out=rb[0:DK, :],
                            in_=_bcast_rows(rdram[None, :], DK))
        nc.vector.scalar_tensor_tensor(
            out=oc_t[jt][r0:r0 + 64, q0:q0 + 1024],
            in0=avsb[0:DK, :], scalar=1.0, in1=rb[0:DK, :],
            op0=MULT, op1=MULT)
        while todo:
            todo.pop(0)[1]()

    # ---- remaining out-projection (qh1 rows) ----
    for qt in range(8, NKT):
        outproj(qt, tail=True)()


_CACHE = {}


def _build():
    if "nc" in _CACHE:
        return _CACHE["nc"]
    from contextlib import ExitStack

    nc = bacc.Bacc("TRN2", target_bir_lowering=False, debug=False,
                   num_devices=NCORES)
    with tile.TileContext(nc) as tc:
        with ExitStack() as ctx:
            _emit(nc, tc, ctx)
    nc.compile()
    _CACHE["nc"] = nc
    return nc


def _stage_segs(xT, nseg, tseg):
    """[D, S] (d = n*128+p) -> [nseg, 128p, 8n, tseg] contiguous."""
    A = xT.reshape(NDT, 128, nseg, tseg)
    return np.ascontiguousarray(A.transpose(2, 1, 0, 3))


def _stage_w(W):
    """[D, J] (d = n*128+p) -> [128p, n, J] contiguous."""
    A = W.reshape(-1, 128, W.shape[1])
    return np.ascontiguousarray(A.transpose(1, 0, 2))


def make_in_maps(query, key, value, Wq, bq, Wk, bk, Wv, bv, Wo, bo):
    import ml_dtypes
    bf = ml_dtypes.bfloat16
    f32 = np.float32
    query = np.asarray(query, f32)
    key = np.asarray(key, f32)
    value = np.asarray(value, f32)
    Wq, Wk, Wv, Wo = (np.asarray(a, f32) for a in (Wq, Wk, Wv, Wo))
    bq, bk, bv = (np.asarray(a, f32) for a in (bq, bk, bv))
    scale = f32(1.0 / np.sqrt(DK))
    xT = {}
    for b in range(B):
        xT[b] = (
            _stage_segs(query[b].T.astype(bf), 4, 512),
            _stage_segs(key[b].T.astype(bf), 4, 512),
            _stage_segs(value[b].T.astype(bf), 8, 256),
        )
    in_maps = []
    for c in range(NCORES):
        b, hh = divmod(c, 2)
        js = slice(hh * DH, (hh + 1) * DH)
        xqT, xkT, xvT = xT[b]
        in_maps.append({
            "xq": xqT,
            "xk": xkT,
            "xv": xvT,
            "wq": _stage_w((Wq[:, js] * scale).astype(bf)),
            "bq": np.ascontiguousarray((bq[js] * scale).reshape(NJT, 128).T),
            "wk": _stage_w(Wk[:, js].astype(bf)),
            "bk": np.ascontiguousarray(bk[js].reshape(NJT, 128).T),
            "wv": _stage_w(Wv[:, js].astype(bf)),
            "bv": np.ascontiguousarray(np.broadcast_to(bv[js], (128, DH))),
            "wo": _stage_w(Wo[js, :].astype(bf)),
        })
    return in_maps


LAST_RESULTS = None


def kernel(query, key, value, Wq, bq, Wk, bk, Wv, bv, Wo, bo):
    global LAST_RESULTS
    import os
    from concourse.bass_utils import run_bass_kernel_spmd

    nc = _build()
    in_maps = make_in_maps(query, key, value, Wq, bq, Wk, bk, Wv, bv, Wo, bo)
    trace = bool(int(os.environ.get("KERNEL_TRACE", "0")))
    res = run_bass_kernel_spmd(nc, in_maps, list(range(NCORES)), trace=trace)
    LAST_RESULTS = res
    bo32 = np.asarray(bo, dtype=np.float32)
    out = np.empty((B, S, D), dtype=np.float32)
    for b in range(B):
        out[b] = (res.results[2 * b]["y"].astype(np.float32)
                  + res.results[2 * b + 1]["y"].astype(np.float32) + bo32)
    return out
